# revision 72
# baseline (speedup 1.0000x reference)
"""Trainium2 Bass kernel for GQA attention with RoPE (dense transformer block).

Strategy (8-way tensor parallel over heads, per the sharding hint):
  - Each of the 8 NeuronCores gets 4 Q heads + 1 KV head (KV head c, Q heads
    4c..4c+3); host sums the 8 partial bf16 outputs (the "all-reduce after wo").
  - All matmul operands are bf16 (fp32 PSUM accumulation). Host pre-packs
    weights so every DMA is wide and contiguous.
  - Causal attention computes the exact block triangle: the diagonal 512x512
    block runs first as one row-strip per KV tile covering only the
    surviving query columns, so each o_ps PSUM bank sees a single
    accumulation group; off-diagonal KV tiles then accumulate at full
    512-query width. Softmax is unnormalized exp; denominators come from
    two alternating elementwise accumulators (DVE) + one GPSIMD
    partition_all_reduce per (batch, head, chunk). o_ps is staged to SBUF
    right after its accumulation stops (alternating ACT/DVE) so its single
    PSUM bank recycles long before the normalize chain completes.
  - The LAST two QKV chunks leave phase A: the second-to-last runs on
    phase B's own PSUM tags (so attention's first tiles wait on per-bank
    drains, not a whole-pool barrier) and the last is pre-seeded as 24
    single-bank filler units (one weight block over 8 k-tiles each,
    accumulation group spanning units) popped one per head through early
    phase B - ready PE work exactly where the attention pipeline is
    otherwise filler-starved; their RoPE is deferred into phase B.
  - x is packed chunk-major in DRAM so one DMA loads 4 k-tiles of a chunk;
    a packed "warm" DRAM tensor (first x tile + k=0 weight slices) gates
    the first matmul on a single DMA while dummy matmuls burn the PE
    p-state ramp; weight/cos/sin/wo loads are staged just-in-time around
    the xt stream, wo loads gated against sequencer hoisting; chunk 0's
    RoPE is deferred until its cos/sin tables land during chunk 1.
  - The wo projection is split into half-token-tile units emitted with a
    delay so a deep ready-pool of wo matmuls fills the PE gaps left by the
    scores->exp->PV dependency chain; the final token tile flushes in
    smaller units so the last PSUM->SBUF->DRAM drain is short.
"""
import math

import numpy as np
import ml_dtypes

import concourse.bass as bass
import concourse.tile as tile
from concourse import bacc, bass_isa, mybir
from concourse.bass_utils import run_bass_kernel_spmd
from concourse.masks import make_identity

B, S, DIM = 2, 2048, 4096
NH, NKV, HD = 32, 8, 128
BS = B * S
NCORES = 8
QH = NH // NCORES          # 4 Q heads per core
DQ = QH * HD               # 512
TCH = 512                  # token chunk
NCH = BS // TCH            # 8 chunks
NKT = DIM // 128           # 32 contraction tiles
P = 128

F32 = mybir.dt.float32
BF16 = mybir.dt.bfloat16
AF = mybir.ActivationFunctionType
NPBF = ml_dtypes.bfloat16

_prog_cache = {}
LAST_RESULTS = None


def _build(variant):
    """variant: 'causal' | 'none' | 'general'"""
    nc = bacc.Bacc(None, target_bir_lowering=False)
    # xTp packed chunk-major: col = tcn*(NKT*TCH) + k*TCH + t, so one DMA
    # loads several consecutive k-tiles of the same token chunk
    xT = nc.dram_tensor("xT", [P, NCH * NKT * TCH], BF16,
                        kind="ExternalInput")
    warm = nc.dram_tensor("warm", [P, TCH + 6 * HD], BF16,
                          kind="ExternalInput")
    wq = nc.dram_tensor("wq", [P, QH * NKT * HD], BF16, kind="ExternalInput")
    wk = nc.dram_tensor("wk", [P, NKT * HD], BF16, kind="ExternalInput")
    wv = nc.dram_tensor("wv", [P, NKT * HD], BF16, kind="ExternalInput")
    wo = nc.dram_tensor("wo", [DQ, DIM], BF16, kind="ExternalInput")
    cosT = nc.dram_tensor("cosT", [64, S], BF16, kind="ExternalInput")
    sinT = nc.dram_tensor("sinT", [64, S], BF16, kind="ExternalInput")
    tri = None
    emask = None
    if variant == "causal":
        tri = nc.dram_tensor("tri", [P, P], BF16, kind="ExternalInput")
    elif variant == "general":
        emask = nc.dram_tensor("emaskT", [S, S], BF16, kind="ExternalInput")
    part = nc.dram_tensor("part", [BS, DIM], BF16, kind="ExternalOutput")

    with tile.TileContext(nc) as tc:
        with (
            tc.tile_pool(name="const", bufs=1) as constp,
            tc.tile_pool(name="big", bufs=1) as bigp,
        ):
            warm_sb = constp.tile([P, TCH + 6 * HD], BF16)
            ident = constp.tile([P, P], BF16)
            make_identity(nc, ident)
            cos_sb = constp.tile([64, S], BF16)
            sin_sb = constp.tile([64, S], BF16)
            tri_sb = None
            if variant == "causal":
                tri_sb = constp.tile([P, P], BF16)

            # persistent per-batch activations (bf16)
            KT_sb = [bigp.tile([P, S], BF16, name=f"KT{b}") for b in range(B)]
            Vtok = [bigp.tile([P, S], BF16, name=f"Vtok{b}") for b in range(B)]
            qt_sb = [[bigp.tile([P, S], BF16, name=f"qt{h}_{b}")
                      for b in range(B)] for h in range(QH)]
            w_sb = [bigp.tile([P, NKT * HD], BF16, name=f"w{m}")
                    for m in range(6)]
            wo_sb = [bigp.tile([P, DIM], BF16, name=f"wo{kk}")
                     for kk in range(4)]

            wsrc = [wq[:, m * NKT * HD:(m + 1) * NKT * HD] for m in range(QH)]
            wsrc += [wk[:, :], wv[:, :]]

            # B-phase SBUF pools open FIRST so their tiles never alias the
            # phase-A scratch (whose release depends on the tcn7 rope tail)
            ebp_cm = tc.tile_pool(name="ebp", bufs=1)
            ebp = ebp_cm.__enter__()
            mkp_cm = tc.tile_pool(name="mkp", bufs=1)
            mkp = mkp_cm.__enter__()
            obp_cm = tc.tile_pool(name="obp", bufs=1)
            obp = obp_cm.__enter__()

            deferred_rope = []
            # rp stays open through phase B (tcn7 rope muls are deferred
            # there so the transition DVE queue is clear for attention)
            rp_cm = tc.tile_pool(name="rp", bufs=1)
            rp = rp_cm.__enter__()
            xtp_cm = tc.tile_pool(name="xtp", bufs=1)
            xtp = xtp_cm.__enter__()
            # ---------------- Phase A: QKV projection + RoPE ----------------
            with (
                tc.tile_pool(name="psA", bufs=1, space="PSUM") as psA,
            ):
                nc.sync.dma_start(warm_sb[:], warm[:, :])
                # p-state spin-up: burn the PE clock ramp on dummy matmuls
                # while the first DMA is in flight
                spin = ebp.tile([P, TCH], BF16, tag="et", bufs=5,
                                name="spin")
                nc.vector.memset(spin[:], 0.0)
                sps = psA.tile([P, TCH], F32, tag="acc", bufs=7,
                               name="spin_ps")
                for _ in range(6):
                    nc.tensor.matmul(sps[:], spin[:, 0:P], spin[:],
                                     start=True, stop=True)
                xt_map = {}

                def load_xt(t_, k0, nk):
                    t = xtp.tile([P, nk * TCH], BF16, tag="xt",
                                 bufs=3 if variant == "causal" else 2,
                                 name=f"xt_{t_}_{k0}")
                    nc.sync.dma_start(
                        t[:], xT[:, t_ * NKT * TCH + k0 * TCH:
                                 t_ * NKT * TCH + (k0 + nk) * TCH])
                    for kk_ in range(k0, k0 + nk):
                        xt_map[(t_, kk_)] = t[:, (kk_ - k0) * TCH:
                                              (kk_ - k0 + 1) * TCH]

                for tcn in range(NCH - 2):
                    b, cb = divmod(tcn, NCH // B)
                    acc = [psA.tile([P, TCH], F32, tag="acc", bufs=7,
                                    name=f"acc{m}_{tcn}") for m in range(6)]
                    if tcn == 0:
                        load_xt(0, 1, 3)
                    for k in range(NKT):
                        if tcn == 0 and k == 0:
                            xsrc = warm_sb[:, 0:TCH]
                            wslc = [warm_sb[:, TCH + m * HD:TCH + (m + 1) * HD]
                                    for m in range(6)]
                        else:
                            xsrc = xt_map[(tcn, k)]
                            wslc = [w_sb[m][:, k * HD:(k + 1) * HD]
                                    for m in range(6)]
                        # xt prefetch: next 4-k tile; next chunk's first
                        # tile is requested from inside this chunk's tail
                        if k % 4 == 0 and k <= NKT - 8:
                            load_xt(tcn, k + 4, 4)
                        if k == 28 and tcn < NCH - 1:
                            load_xt(tcn + 1, 0, 4)
                        # staged just-in-time weight loads; cos/sin/tri
                        # land late in chunk 0 (needed only at its drain)
                        wstage = {1: (0, 4), 2: (4, 10), 3: (10, 16),
                                  5: (16, 24), 10: (24, 32)}
                        if tcn == 0 and k in wstage:
                            lo, hi = wstage[k]
                            for m in range(6):
                                eng = nc.gpsimd if m % 2 == 0 else nc.sync
                                eng.dma_start(w_sb[m][:, lo * HD:hi * HD],
                                              wsrc[m][:, lo * HD:hi * HD])
                        if tcn == 1 and k in (0, 1):
                            src = (cos_sb, cosT) if k == 0 else (sin_sb, sinT)
                            nc.gpsimd.dma_start(src[0][:, :], src[1][:, :])
                        if tcn == 1 and k == 2 and variant == "causal":
                            nc.gpsimd.dma_start(tri_sb[:], tri[:, :])
                        if tcn == 1 and k == 3 and deferred_rope:
                            deferred_rope.pop(0)()
                        if tcn == 2 and k == 0:
                            # dispatch gate: Pool SEQ would otherwise hoist
                            # the dep-free wo loads into chunk 0's DMA crunch
                            gate = rp.tile([P, 16], BF16, tag="gate", bufs=1,
                                           name="gate")
                            nc.gpsimd.tensor_copy(gate[:], KT_sb[0][:, 0:16])
                        if tcn in (2, 3) and k in (0, 8, 16, 24):
                            kk = (tcn - 2) * 2 + (k // 16)
                            hf = (k // 8) % 2
                            nc.gpsimd.dma_start(
                                wo_sb[kk][:, hf * 2048:(hf + 1) * 2048],
                                wo[kk * P:(kk + 1) * P,
                                   hf * 2048:(hf + 1) * 2048])

                        for m in range(6):
                            nc.tensor.matmul(
                                acc[m][:], wslc[m], xsrc,
                                start=(k == 0), stop=(k == NKT - 1))

                    # ---- drain: all copies first, rope muls after ----
                    vch = rp.tile([P, TCH], BF16, tag="vch", bufs=2,
                                  name=f"vch_{tcn}")
                    nc.scalar.copy(vch[:], acc[5][:])
                    tp4 = psA.tile([P, TCH], BF16, tag="tp", bufs=1,
                                   name=f"tp_{tcn}")
                    for j in range(4):
                        nc.tensor.transpose(
                            tp4[:, j * P:(j + 1) * P],
                            vch[:, j * P:(j + 1) * P], ident[:])
                    nc.vector.tensor_copy(
                        Vtok[b][:, cb * TCH:(cb + 1) * TCH], tp4[:])
                    cs = cos_sb[:, cb * TCH:(cb + 1) * TCH]
                    sn = sin_sb[:, cb * TCH:(cb + 1) * TCH]
                    morder = list(range(5))
                    slos, shis = {}, {}
                    for i, m in enumerate(morder):
                        slo = rp.tile([64, TCH], BF16, tag="slo", bufs=5,
                                      name=f"slo{m}_{tcn}")
                        shi = rp.tile([64, TCH], BF16, tag="shi", bufs=5,
                                      name=f"shi{m}_{tcn}")
                        if i % 2 == 0:
                            nc.scalar.copy(slo[:], acc[m][0:64, :])
                            nc.vector.tensor_copy(shi[:], acc[m][64:P, :])
                        else:
                            nc.vector.tensor_copy(slo[:], acc[m][0:64, :])
                            nc.scalar.copy(shi[:], acc[m][64:P, :])
                        slos[m] = slo
                        shis[m] = shi
                    def rope_muls(b=b, cb=cb, tcn=tcn, slos=slos,
                                  shis=shis, cs=cs, sn=sn):
                      for m in range(5):
                        slo, shi = slos[m], shis[m]
                        dst = qt_sb[m][b] if m < QH else KT_sb[b]
                        o_lo = dst[0:64, cb * TCH:(cb + 1) * TCH]
                        o_hi = dst[64:P, cb * TCH:(cb + 1) * TCH]
                        tA = rp.tile([64, TCH], BF16, tag="tA", bufs=1,
                                     name=f"tA{m}_{tcn}")
                        tB = rp.tile([64, TCH], BF16, tag="tB", bufs=1,
                                     name=f"tB{m}_{tcn}")
                        nc.vector.tensor_mul(tA[:], slo[:], cs)
                        nc.vector.tensor_mul(tB[:], shi[:], sn)
                        nc.vector.tensor_sub(o_lo, tA[:], tB[:])
                        tC = rp.tile([64, TCH], BF16, tag="tC", bufs=1,
                                     name=f"tC{m}_{tcn}")
                        tD = rp.tile([64, TCH], BF16, tag="tD", bufs=1,
                                     name=f"tD{m}_{tcn}")
                        nc.vector.tensor_mul(tC[:], slo[:], sn)
                        nc.vector.tensor_mul(tD[:], shi[:], cs)
                        nc.vector.tensor_add(o_hi, tC[:], tD[:])
                    if tcn == 0:
                        deferred_rope.append(rope_muls)
                    else:
                        rope_muls()

            # ------------- Phase B: attention + wo, per (batch, chunk) ------
            # PSUM creation order: sc(3 banks), pp(3), o(2) - lands on the
            # phase-A banks in the order they are drained.
            with (
                tc.tile_pool(name="psB", bufs=1, space="PSUM") as psB,
            ):
                # ---- tcn6: last QKV chunk runs with accumulators on
                # phase-B PSUM tags, so attention's first tiles wait on
                # per-bank drains instead of a whole-pool barrier ----
                t6 = NCH - 2
                b6, cb6 = divmod(t6, NCH // B)
                acc6 = [psB.tile([P, TCH], F32, tag="sc", bufs=3,
                                 name=f"a6_{m}") for m in range(3)]
                acc6 += [psB.tile([P, TCH], F32, tag="pp", bufs=3,
                                  name=f"a6_{m}") for m in range(3, 6)]
                for k in range(NKT):
                    if k % 4 == 0 and k <= NKT - 8:
                        load_xt(t6, k + 4, 4)
                    if k == 28:
                        load_xt(t6 + 1, 0, 4)
                    if k == 30:
                        load_xt(t6 + 1, 4, 4)
                    for m in range(6):
                        nc.tensor.matmul(
                            acc6[m][:], w_sb[m][:, k * HD:(k + 1) * HD],
                            xt_map[(t6, k)],
                            start=(k == 0), stop=(k == NKT - 1))
                vch6 = rp.tile([P, TCH], BF16, tag="vch", bufs=2,
                               name="vch_6")
                nc.scalar.copy(vch6[:], acc6[5][:])
                tp6 = psB.tile([P, TCH], BF16, tag="a7", bufs=1, name="tp_6")
                for jj in range(4):
                    nc.tensor.transpose(tp6[:, jj * P:(jj + 1) * P],
                                        vch6[:, jj * P:(jj + 1) * P],
                                        ident[:])
                nc.vector.tensor_copy(
                    Vtok[b6][:, cb6 * TCH:(cb6 + 1) * TCH], tp6[:])
                qk6 = {}
                for m in range(5):
                    slo = rp.tile([64, TCH], BF16, tag="slo", bufs=5,
                                  name=f"slo{m}_6")
                    shi = rp.tile([64, TCH], BF16, tag="shi", bufs=5,
                                  name=f"shi{m}_6")
                    if m % 2 == 0:
                        nc.scalar.copy(slo[:], acc6[m][0:64, :])
                        nc.vector.tensor_copy(shi[:], acc6[m][64:P, :])
                    else:
                        nc.vector.tensor_copy(slo[:], acc6[m][0:64, :])
                        nc.scalar.copy(shi[:], acc6[m][64:P, :])
                    qk6[m] = (slo, shi)

                def rope6():
                    cs = cos_sb[:, cb6 * TCH:(cb6 + 1) * TCH]
                    sn = sin_sb[:, cb6 * TCH:(cb6 + 1) * TCH]
                    for m in range(5):
                        slo, shi = qk6[m]
                        dst = qt_sb[m][b6] if m < QH else KT_sb[b6]
                        o_lo = dst[0:64, cb6 * TCH:(cb6 + 1) * TCH]
                        o_hi = dst[64:P, cb6 * TCH:(cb6 + 1) * TCH]
                        tA = rp.tile([64, TCH], BF16, tag="tA", bufs=1,
                                     name=f"tA{m}_6")
                        tB = rp.tile([64, TCH], BF16, tag="tB", bufs=1,
                                     name=f"tB{m}_6")
                        nc.vector.tensor_mul(tA[:], slo[:], cs)
                        nc.vector.tensor_mul(tB[:], shi[:], sn)
                        nc.vector.tensor_sub(o_lo, tA[:], tB[:])
                        tC = rp.tile([64, TCH], BF16, tag="tC", bufs=1,
                                     name=f"tC{m}_6")
                        tD = rp.tile([64, TCH], BF16, tag="tD", bufs=1,
                                     name=f"tD{m}_6")
                        nc.vector.tensor_mul(tC[:], slo[:], sn)
                        nc.vector.tensor_mul(tD[:], shi[:], cs)
                        nc.vector.tensor_add(o_hi, tC[:], tD[:])
                deferred_rope.append(rope6)

                pending = []
                mpend = []
                a7_state = {}

                def make_mpass(m, j):
                    # deferred tcn7 QKV: weight block m over k-tiles
                    # 8j..8j+7 into one PSUM bank (group spans units)
                    def emit():
                        tcn = NCH - 1
                        cb = 3
                        if j == 0:
                            a7_state[m] = psB.tile(
                                [P, TCH], F32, tag="a7", bufs=1,
                                name=f"a7_{m}")
                        acc7 = a7_state[m]
                        for kk in range(8 * j, 8 * j + 8):
                            if (tcn, kk) not in xt_map:
                                load_xt(tcn, kk - kk % 4, 4)
                            nc.tensor.matmul(
                                acc7[:], w_sb[m][:, kk * HD:(kk + 1) * HD],
                                xt_map[(tcn, kk)],
                                start=(kk == 0), stop=(kk == NKT - 1))
                        if j < 3:
                            return
                        for kk in range(NKT):
                            xt_map.pop((tcn, kk), None)
                        cs = cos_sb[:, cb * TCH:(cb + 1) * TCH]
                        sn = sin_sb[:, cb * TCH:(cb + 1) * TCH]
                        if m == 5:
                            vch = rp.tile([P, TCH], BF16, tag="vch", bufs=2,
                                          name="vch_7")
                            nc.scalar.copy(vch[:], acc7[:])
                            tp4 = psB.tile([P, TCH], BF16, tag="a7", bufs=1,
                                           name="tp_7")
                            for jj in range(4):
                                nc.tensor.transpose(
                                    tp4[:, jj * P:(jj + 1) * P],
                                    vch[:, jj * P:(jj + 1) * P], ident[:])
                            nc.vector.tensor_copy(
                                Vtok[1][:, cb * TCH:(cb + 1) * TCH], tp4[:])
                            return
                        slo = rp.tile([64, TCH], BF16, tag="slo", bufs=5,
                                      name=f"slo{m}_7")
                        shi = rp.tile([64, TCH], BF16, tag="shi", bufs=5,
                                      name=f"shi{m}_7")
                        if m % 2 == 0:
                            nc.scalar.copy(slo[:], acc7[0:64, :])
                            nc.vector.tensor_copy(shi[:], acc7[64:P, :])
                        else:
                            nc.vector.tensor_copy(slo[:], acc7[0:64, :])
                            nc.scalar.copy(shi[:], acc7[64:P, :])
                        dst = qt_sb[m][1] if m < QH else KT_sb[1]
                        o_lo = dst[0:64, cb * TCH:(cb + 1) * TCH]
                        o_hi = dst[64:P, cb * TCH:(cb + 1) * TCH]
                        tA = rp.tile([64, TCH], BF16, tag="tA", bufs=1,
                                     name=f"tA{m}_7")
                        tB = rp.tile([64, TCH], BF16, tag="tB", bufs=1,
                                     name=f"tB{m}_7")
                        nc.vector.tensor_mul(tA[:], slo[:], cs)
                        nc.vector.tensor_mul(tB[:], shi[:], sn)
                        nc.vector.tensor_sub(o_lo, tA[:], tB[:])
                        tC = rp.tile([64, TCH], BF16, tag="tC", bufs=1,
                                     name=f"tC{m}_7")
                        tD = rp.tile([64, TCH], BF16, tag="tD", bufs=1,
                                     name=f"tD{m}_7")
                        nc.vector.tensor_mul(tC[:], slo[:], sn)
                        nc.vector.tensor_mul(tD[:], shi[:], cs)
                        nc.vector.tensor_add(o_hi, tC[:], tD[:])
                    return emit

                for m_ in (5, 4, 0, 1, 2, 3):
                    for j_ in range(4):
                        mpend.append(make_mpass(m_, j_))

                def make_unit(b, sc, o_g, tj, half, njs):
                    def emit():
                        tt = 4 * sc + tj
                        w0 = njs[0] * TCH
                        w1 = (njs[-1] + 1) * TCH
                        tag = "ob" if len(njs) > 1 else "obf"
                        ob = obp.tile([P, w1 - w0], BF16, tag=tag,
                                      bufs=2 if tag == "ob" else 2,
                                      name=f"ob_{b}_{tt}_{half}")
                        for i, nn in enumerate(njs):
                            pp = psB.tile([P, TCH], F32, tag="pp", bufs=3,
                                          name=f"pp_{b}_{tt}_{nn}")
                            for kk in range(4):
                                nc.tensor.matmul(
                                    pp[:],
                                    o_g[kk][:, tj * P:(tj + 1) * P],
                                    wo_sb[kk][:, nn * TCH:(nn + 1) * TCH],
                                    start=(kk == 0), stop=(kk == 3))
                            dst = ob[:, i * TCH:(i + 1) * TCH]
                            if nn % 2 == 0:
                                nc.scalar.copy(dst, pp[:])
                            else:
                                nc.vector.tensor_copy(dst, pp[:])
                        nc.sync.dma_start(
                            part[b * S + tt * P:b * S + (tt + 1) * P, w0:w1],
                            ob[:])
                    return emit

                for b in range(B):
                    for sc in (0, 1, 2, 3):
                        nod = 4 * sc if variant == "causal" else 16
                        o_g = [obp.tile([P, TCH], BF16, tag=f"og{h}",
                                        bufs=3, name=f"og_{b}_{sc}_{h}")
                               for h in range(QH)]
                        for h in range(QH):
                            if mpend:
                                mpend.pop(0)()
                            for _ in range(2):
                                if len(pending) > 4:
                                    pending.pop(0)()
                            o_ps = psB.tile([P, TCH], F32, tag="o", bufs=1,
                                            name=f"o_{b}_{sc}_{h}")
                            E_ab = [ebp.tile([P, TCH], BF16, tag=f"ea{par}",
                                             bufs=2,
                                             name=f"ea{par}_{b}_{sc}_{h}")
                                    for par in range(2)]
                            qts = qt_sb[h][b][:, sc * TCH:(sc + 1) * TCH]
                            if variant == "causal":
                                # diagonal 512x512 block first: strip d covers
                                # query cols d*128..512, so the d=0 strip
                                # opens the o_ps accumulation group full-width
                                for d in range(4):
                                    w = (4 - d) * P
                                    c0 = d * P
                                    scd = psB.tile(
                                        [P, TCH], F32, tag="sc", bufs=3,
                                        name=f"sd_{b}_{sc}_{h}_{d}")
                                    nc.tensor.matmul(
                                        scd[:, 0:w],
                                        KT_sb[b][:,
                                                 (nod + d) * P:
                                                 (nod + d + 1) * P],
                                        qts[:, c0:TCH],
                                        start=True, stop=True)
                                    etd = ebp.tile(
                                        [P, TCH], BF16, tag="etd", bufs=3,
                                        name=f"ed_{b}_{sc}_{h}_{d}")
                                    nc.scalar.activation(etd[:, 0:w],
                                                         scd[:, 0:w], AF.Exp)
                                    nc.vector.tensor_mul(
                                        etd[:, 0:P], etd[:, 0:P], tri_sb[:])
                                    ea = E_ab[d % 2]
                                    if d < 2:
                                        nc.vector.tensor_copy(
                                            ea[:, c0:TCH], etd[:, 0:w])
                                    else:
                                        nc.vector.tensor_add(
                                            ea[:, c0:TCH], ea[:, c0:TCH],
                                            etd[:, 0:w])
                                    nc.tensor.matmul(
                                        o_ps[:, c0:TCH],
                                        Vtok[b][:, (nod + d) * P:
                                                (nod + d + 1) * P],
                                        etd[:, 0:w],
                                        start=(d == 0),
                                        stop=(nod == 0 and d == 3))
                            for tt in range(nod):
                                sc_ps = psB.tile([P, TCH], F32, tag="sc",
                                                 bufs=3,
                                                 name=f"s_{b}_{sc}_{h}_{tt}")
                                nc.tensor.matmul(
                                    sc_ps[:],
                                    KT_sb[b][:, tt * P:(tt + 1) * P],
                                    qts, start=True, stop=True)
                                et = ebp.tile([P, TCH], BF16, tag="et",
                                              bufs=5,
                                              name=f"et_{b}_{sc}_{h}_{tt}")
                                if variant == "general":
                                    etm = ebp.tile(
                                        [P, TCH], BF16, tag="etm", bufs=3,
                                        name=f"em_{b}_{sc}_{h}_{tt}")
                                    nc.scalar.activation(etm[:], sc_ps[:],
                                                         AF.Exp)
                                    mg = mkp.tile(
                                        [P, TCH], BF16, tag="mg", bufs=3,
                                        name=f"mg_{b}_{sc}_{h}_{tt}")
                                    nc.sync.dma_start(
                                        mg[:],
                                        emask[tt * P:(tt + 1) * P,
                                              sc * TCH:(sc + 1) * TCH])
                                    nc.vector.tensor_mul(et[:], etm[:], mg[:])
                                else:
                                    nc.scalar.activation(et[:], sc_ps[:],
                                                         AF.Exp)
                                if variant == "causal":
                                    ea = E_ab[tt % 2]
                                    if tt == 1:
                                        # ea1 cols 0:128 not covered by the
                                        # d=1 diagonal strip copy
                                        nc.vector.tensor_copy(
                                            ea[:, 0:P], et[:, 0:P])
                                        nc.vector.tensor_add(
                                            ea[:, P:TCH], ea[:, P:TCH],
                                            et[:, P:TCH])
                                    else:
                                        nc.vector.tensor_add(ea[:], ea[:],
                                                             et[:])
                                else:
                                    ea = E_ab[tt % 2]
                                    if tt < 2:
                                        nc.vector.tensor_copy(ea[:], et[:])
                                    else:
                                        nc.vector.tensor_add(ea[:], ea[:],
                                                             et[:])
                                nc.tensor.matmul(
                                    o_ps[:], Vtok[b][:, tt * P:(tt + 1) * P],
                                    et[:], start=(variant != "causal"
                                                  and tt == 0),
                                    stop=(tt == nod - 1))
                            ost = obp.tile([P, TCH], BF16, tag="ost",
                                           bufs=2, name=f"ost_{b}_{sc}_{h}")
                            if h % 2 == 0:
                                nc.scalar.copy(ost[:], o_ps[:])
                            else:
                                nc.vector.tensor_copy(ost[:], o_ps[:])
                            e_sum = ebp.tile([P, TCH], BF16, tag="es",
                                             bufs=1, name=f"es_{b}_{sc}_{h}")
                            if variant == "causal" and nod == 0:
                                nc.vector.tensor_copy(e_sum[:, 0:P],
                                                      E_ab[0][:, 0:P])
                                nc.vector.tensor_add(e_sum[:, P:TCH],
                                                     E_ab[0][:, P:TCH],
                                                     E_ab[1][:, P:TCH])
                            else:
                                nc.vector.tensor_add(e_sum[:], E_ab[0][:],
                                                     E_ab[1][:])
                            srec = obp.tile([P, TCH], F32, tag="sr", bufs=1,
                                            name=f"sr_{b}_{sc}_{h}")
                            nc.gpsimd.partition_all_reduce(
                                srec[:], e_sum[:], P, bass_isa.ReduceOp.add)
                            rec = obp.tile([P, TCH], BF16, tag="rec", bufs=1,
                                           name=f"rec_{b}_{sc}_{h}")
                            with nc.allow_low_precision(
                                    reason="softmax denom, 2e-2 gate"):
                                nc.vector.reciprocal(rec[:], srec[:])
                            nc.vector.tensor_mul(o_g[h][:], ost[:], rec[:])

                        if b == 0 and sc == 0:
                            for fn_ in deferred_rope:
                                fn_()
                            deferred_rope = []
                        last = (b == B - 1 and sc == 3)
                        for tj in range(4):
                            if last and tj == 3:
                                for u, njs in enumerate(
                                        ([0, 1], [2, 3], [4, 5], [6], [7])):
                                    pending.append(
                                        make_unit(b, sc, o_g, tj, u, njs))
                            else:
                                for half in range(2):
                                    pending.append(make_unit(
                                        b, sc, o_g, tj, half,
                                        [4 * half + i for i in range(4)]))
                for fn_ in pending:
                    fn_()
                pending = []

                xtp_cm.__exit__(None, None, None)
                rp_cm.__exit__(None, None, None)
                obp_cm.__exit__(None, None, None)
                mkp_cm.__exit__(None, None, None)
                ebp_cm.__exit__(None, None, None)

    nc.compile()
    return nc


def _get_prog(variant):
    if variant not in _prog_cache:
        _prog_cache[variant] = _build(variant)
    return _prog_cache[variant]


def prepare(inputs):
    """Host-side sharding prep: returns (variant, program, per-core input maps)."""
    x = np.asarray(inputs["x"], dtype=np.float32)
    wq = np.asarray(inputs["wq"], dtype=np.float32)
    wk = np.asarray(inputs["wk"], dtype=np.float32)
    wv = np.asarray(inputs["wv"], dtype=np.float32)
    wo = np.asarray(inputs["wo"], dtype=np.float32)
    fc = np.asarray(inputs["freqs_cos"], dtype=np.float32)
    fs = np.asarray(inputs["freqs_sin"], dtype=np.float32)
    mask = np.asarray(inputs["mask"], dtype=np.float32)

    xx = x.reshape(BS, DIM)
    # packed chunk-major: xT[p, tcn*(NKT*TCH) + k*TCH + t] = x[tcn*TCH+t, k*P+p]
    xT = np.ascontiguousarray(
        xx.reshape(NCH, TCH, NKT, P).transpose(3, 0, 2, 1)
        .reshape(P, NCH * NKT * TCH)).astype(NPBF)
    warm_x = np.ascontiguousarray(xx[0:TCH, 0:P].T).astype(NPBF)
    perm = np.concatenate([np.arange(0, HD, 2), np.arange(1, HD, 2)])
    wq_p = (wq.reshape(DIM, NH, HD)[:, :, perm] / math.sqrt(HD))
    wk_p = wk.reshape(DIM, NKV, HD)[:, :, perm]
    cosT = np.ascontiguousarray(fc.T).astype(NPBF)
    sinT = np.ascontiguousarray(fs.T).astype(NPBF)

    if not mask.any():
        variant = "none"
    else:
        il, jl = np.tril_indices(S)
        iu, ju = np.triu_indices(S, 1)
        if np.all(mask[il, jl] == 0.0) and np.all(mask[iu, ju] <= -1e8):
            variant = "causal"
        else:
            variant = "general"

    tri128 = None
    emaskT = None
    if variant == "causal":
        t = np.arange(P)[:, None]
        q = np.arange(P)[None, :]
        tri128 = (q >= t).astype(NPBF)
    elif variant == "general":
        with np.errstate(under="ignore", over="ignore"):
            emaskT = np.ascontiguousarray(np.exp(mask).T).astype(NPBF)

    nc = _get_prog(variant)

    in_maps = []
    for c in range(NCORES):
        wqc = wq_p[:, c * QH:(c + 1) * QH, :]                    # [DIM,QH,HD]
        wqc = np.ascontiguousarray(
            wqc.reshape(NKT, P, QH, HD).transpose(1, 2, 0, 3)
            .reshape(P, QH * NKT * HD)).astype(NPBF)
        wkc = np.ascontiguousarray(
            wk_p[:, c, :].reshape(NKT, P, HD).transpose(1, 0, 2)
            .reshape(P, NKT * HD)).astype(NPBF)
        wvc = np.ascontiguousarray(
            wv[:, c * HD:(c + 1) * HD].reshape(NKT, P, HD).transpose(1, 0, 2)
            .reshape(P, NKT * HD)).astype(NPBF)
        warm = np.concatenate(
            [warm_x]
            + [wqc[:, m * NKT * HD:m * NKT * HD + HD] for m in range(QH)]
            + [wkc[:, 0:HD], wvc[:, 0:HD]], axis=1)
        m = {
            "xT": xT,
            "warm": np.ascontiguousarray(warm).astype(NPBF),
            "wq": wqc,
            "wk": wkc,
            "wv": wvc,
            "wo": np.ascontiguousarray(
                wo[c * DQ:(c + 1) * DQ, :]).astype(NPBF),
            "cosT": cosT,
            "sinT": sinT,
        }
        if variant == "causal":
            m["tri"] = tri128
        elif variant == "general":
            m["emaskT"] = emaskT
        in_maps.append(m)
    return variant, nc, in_maps


def kernel(**inputs):
    global LAST_RESULTS
    variant, nc, in_maps = prepare(inputs)
    out = None
    for attempt in range(3):
        res = run_bass_kernel_spmd(nc, in_maps, core_ids=list(range(NCORES)))
        LAST_RESULTS = res
        out = np.zeros((BS, DIM), dtype=np.float64)
        ok = True
        for c in range(NCORES):
            p = np.asarray(res.results[c]["part"], dtype=np.float64)
            # flaky-execution guard: a healthy partial is finite, O(1)-scale,
            # and every token row is nonzero (dense projection of dense
            # data); huge values, NaNs, or any all-zero row mean the device
            # produced a bad/partial result -> re-execute
            rowmax = np.abs(p).max(axis=1)
            if not np.isfinite(p).all() or p.max() > 1e3 \
                    or p.min() < -1e3 or rowmax.min() == 0.0:
                ok = False
            out += p
        if ok:
            break
    return out.reshape(B, S, DIM).astype(np.float32)


# revision 74
# speedup vs baseline: 1.0002x; 1.0002x over previous
"""Trainium2 Bass kernel for GQA attention with RoPE (dense transformer block).

Strategy (8-way tensor parallel over heads, per the sharding hint):
  - Each of the 8 NeuronCores gets 4 Q heads + 1 KV head (KV head c, Q heads
    4c..4c+3); host sums the 8 partial bf16 outputs (the "all-reduce after wo").
  - All matmul operands are bf16 (fp32 PSUM accumulation). Host pre-packs
    weights so every DMA is wide and contiguous.
  - Causal attention computes the exact block triangle: the diagonal 512x512
    block runs first as one row-strip per KV tile covering only the
    surviving query columns, so each o_ps PSUM bank sees a single
    accumulation group; off-diagonal KV tiles then accumulate at full
    512-query width. Softmax is unnormalized exp; denominators come from
    two alternating elementwise accumulators (DVE) + one GPSIMD
    partition_all_reduce per (batch, head, chunk). o_ps is staged to SBUF
    right after its accumulation stops (alternating ACT/DVE) so its single
    PSUM bank recycles long before the normalize chain completes.
  - The LAST two QKV chunks leave phase A: the second-to-last runs on
    phase B's own PSUM tags (so attention's first tiles wait on per-bank
    drains, not a whole-pool barrier) and the last is pre-seeded as 24
    single-bank filler units (one weight block over 8 k-tiles each,
    accumulation group spanning units) popped one per head through early
    phase B - ready PE work exactly where the attention pipeline is
    otherwise filler-starved; their RoPE is deferred into phase B.
  - x is packed chunk-major in DRAM so one DMA loads 4 k-tiles of a chunk;
    a packed "warm" DRAM tensor (first x tile + k=0 weight slices) gates
    the first matmul on a single DMA while dummy matmuls burn the PE
    p-state ramp; weight/cos/sin/wo loads are staged just-in-time around
    the xt stream, wo loads gated against sequencer hoisting; chunk 0's
    RoPE is deferred until its cos/sin tables land during chunk 1.
  - The wo projection is split into half-token-tile units emitted with a
    delay so a deep ready-pool of wo matmuls fills the PE gaps left by the
    scores->exp->PV dependency chain; the final token tile flushes in
    smaller units so the last PSUM->SBUF->DRAM drain is short.
"""
import math

import numpy as np
import ml_dtypes

import concourse.bass as bass
import concourse.tile as tile
from concourse import bacc, bass_isa, mybir
from concourse.bass_utils import run_bass_kernel_spmd
from concourse.masks import make_identity

B, S, DIM = 2, 2048, 4096
NH, NKV, HD = 32, 8, 128
BS = B * S
NCORES = 8
QH = NH // NCORES          # 4 Q heads per core
DQ = QH * HD               # 512
TCH = 512                  # token chunk
NCH = BS // TCH            # 8 chunks
NKT = DIM // 128           # 32 contraction tiles
P = 128

F32 = mybir.dt.float32
BF16 = mybir.dt.bfloat16
AF = mybir.ActivationFunctionType
NPBF = ml_dtypes.bfloat16

_prog_cache = {}
LAST_RESULTS = None


def _build(variant):
    """variant: 'causal' | 'none' | 'general'"""
    nc = bacc.Bacc(None, target_bir_lowering=False)
    # xTp packed chunk-major: col = tcn*(NKT*TCH) + k*TCH + t, so one DMA
    # loads several consecutive k-tiles of the same token chunk
    xT = nc.dram_tensor("xT", [P, NCH * NKT * TCH], BF16,
                        kind="ExternalInput")
    warm = nc.dram_tensor("warm", [P, TCH + 6 * HD], BF16,
                          kind="ExternalInput")
    wq = nc.dram_tensor("wq", [P, QH * NKT * HD], BF16, kind="ExternalInput")
    wk = nc.dram_tensor("wk", [P, NKT * HD], BF16, kind="ExternalInput")
    wv = nc.dram_tensor("wv", [P, NKT * HD], BF16, kind="ExternalInput")
    wo = nc.dram_tensor("wo", [DQ, DIM], BF16, kind="ExternalInput")
    cosT = nc.dram_tensor("cosT", [64, S], BF16, kind="ExternalInput")
    sinT = nc.dram_tensor("sinT", [64, S], BF16, kind="ExternalInput")
    tri = None
    emask = None
    if variant == "causal":
        tri = nc.dram_tensor("tri", [P, P], BF16, kind="ExternalInput")
    elif variant == "general":
        emask = nc.dram_tensor("emaskT", [S, S], BF16, kind="ExternalInput")
    part = nc.dram_tensor("part", [BS, DIM], BF16, kind="ExternalOutput")

    with tile.TileContext(nc) as tc:
        with (
            tc.tile_pool(name="const", bufs=1) as constp,
            tc.tile_pool(name="big", bufs=1) as bigp,
        ):
            warm_sb = constp.tile([P, TCH + 6 * HD], BF16)
            ident = constp.tile([P, P], BF16)
            make_identity(nc, ident)
            cos_sb = constp.tile([64, S], BF16)
            sin_sb = constp.tile([64, S], BF16)
            tri_sb = None
            if variant == "causal":
                tri_sb = constp.tile([P, P], BF16)

            # persistent per-batch activations (bf16)
            KT_sb = [bigp.tile([P, S], BF16, name=f"KT{b}") for b in range(B)]
            Vtok = [bigp.tile([P, S], BF16, name=f"Vtok{b}") for b in range(B)]
            qt_sb = [[bigp.tile([P, S], BF16, name=f"qt{h}_{b}")
                      for b in range(B)] for h in range(QH)]
            w_sb = [bigp.tile([P, NKT * HD], BF16, name=f"w{m}")
                    for m in range(6)]
            wo_sb = [bigp.tile([P, DIM], BF16, name=f"wo{kk}")
                     for kk in range(4)]

            wsrc = [wq[:, m * NKT * HD:(m + 1) * NKT * HD] for m in range(QH)]
            wsrc += [wk[:, :], wv[:, :]]

            # B-phase SBUF pools open FIRST so their tiles never alias the
            # phase-A scratch (whose release depends on the tcn7 rope tail)
            ebp_cm = tc.tile_pool(name="ebp", bufs=1)
            ebp = ebp_cm.__enter__()
            mkp_cm = tc.tile_pool(name="mkp", bufs=1)
            mkp = mkp_cm.__enter__()
            obp_cm = tc.tile_pool(name="obp", bufs=1)
            obp = obp_cm.__enter__()

            deferred_rope = []
            # rp stays open through phase B (tcn7 rope muls are deferred
            # there so the transition DVE queue is clear for attention)
            rp_cm = tc.tile_pool(name="rp", bufs=1)
            rp = rp_cm.__enter__()
            xtp_cm = tc.tile_pool(name="xtp", bufs=1)
            xtp = xtp_cm.__enter__()
            # ---------------- Phase A: QKV projection + RoPE ----------------
            with (
                tc.tile_pool(name="psA", bufs=1, space="PSUM") as psA,
            ):
                nc.sync.dma_start(warm_sb[:], warm[:, :])
                # p-state spin-up: burn the PE clock ramp on dummy matmuls
                # while the first DMA is in flight
                spin = ebp.tile([P, TCH], BF16, tag="et", bufs=5,
                                name="spin")
                nc.vector.memset(spin[:], 0.0)
                sps = psA.tile([P, TCH], F32, tag="acc", bufs=7,
                               name="spin_ps")
                for _ in range(6):
                    nc.tensor.matmul(sps[:], spin[:, 0:P], spin[:],
                                     start=True, stop=True)
                xt_map = {}

                def load_xt(t_, k0, nk):
                    t = xtp.tile([P, nk * TCH], BF16, tag="xt",
                                 bufs=3 if variant == "causal" else 2,
                                 name=f"xt_{t_}_{k0}")
                    nc.sync.dma_start(
                        t[:], xT[:, t_ * NKT * TCH + k0 * TCH:
                                 t_ * NKT * TCH + (k0 + nk) * TCH])
                    for kk_ in range(k0, k0 + nk):
                        xt_map[(t_, kk_)] = t[:, (kk_ - k0) * TCH:
                                              (kk_ - k0 + 1) * TCH]

                for tcn in range(NCH - 2):
                    b, cb = divmod(tcn, NCH // B)
                    acc = [psA.tile([P, TCH], F32, tag="acc", bufs=7,
                                    name=f"acc{m}_{tcn}") for m in range(6)]
                    if tcn == 0:
                        load_xt(0, 1, 3)
                    for k in range(NKT):
                        if tcn == 0 and k == 0:
                            xsrc = warm_sb[:, 0:TCH]
                            wslc = [warm_sb[:, TCH + m * HD:TCH + (m + 1) * HD]
                                    for m in range(6)]
                        else:
                            xsrc = xt_map[(tcn, k)]
                            wslc = [w_sb[m][:, k * HD:(k + 1) * HD]
                                    for m in range(6)]
                        # xt prefetch: next 4-k tile; next chunk's first
                        # tile is requested from inside this chunk's tail
                        if k % 4 == 0 and k <= NKT - 8:
                            load_xt(tcn, k + 4, 4)
                        if k == 28 and tcn < NCH - 1:
                            load_xt(tcn + 1, 0, 4)
                        # staged just-in-time weight loads; cos/sin/tri
                        # land late in chunk 0 (needed only at its drain)
                        wstage = {1: (0, 4), 2: (4, 10), 3: (10, 16),
                                  5: (16, 24), 10: (24, 32)}
                        if tcn == 0 and k in wstage:
                            lo, hi = wstage[k]
                            for m in range(6):
                                eng = nc.gpsimd if m % 2 == 0 else nc.sync
                                eng.dma_start(w_sb[m][:, lo * HD:hi * HD],
                                              wsrc[m][:, lo * HD:hi * HD])
                        if tcn == 1 and k in (0, 1):
                            src = (cos_sb, cosT) if k == 0 else (sin_sb, sinT)
                            nc.gpsimd.dma_start(src[0][:, :], src[1][:, :])
                        if tcn == 1 and k == 2 and variant == "causal":
                            nc.gpsimd.dma_start(tri_sb[:], tri[:, :])
                        if tcn == 1 and k == 3 and deferred_rope:
                            deferred_rope.pop(0)()
                        if tcn == 2 and k == 0:
                            # dispatch gate: Pool SEQ would otherwise hoist
                            # the dep-free wo loads into chunk 0's DMA crunch
                            gate = rp.tile([P, 16], BF16, tag="gate", bufs=1,
                                           name="gate")
                            nc.gpsimd.tensor_copy(gate[:], KT_sb[0][:, 0:16])
                        if tcn in (2, 3) and k in (0, 8, 16, 24):
                            kk = (tcn - 2) * 2 + (k // 16)
                            hf = (k // 8) % 2
                            nc.gpsimd.dma_start(
                                wo_sb[kk][:, hf * 2048:(hf + 1) * 2048],
                                wo[kk * P:(kk + 1) * P,
                                   hf * 2048:(hf + 1) * 2048])

                        for m in range(6):
                            nc.tensor.matmul(
                                acc[m][:], wslc[m], xsrc,
                                start=(k == 0), stop=(k == NKT - 1))

                    # ---- drain: all copies first, rope muls after ----
                    vch = rp.tile([P, TCH], BF16, tag="vch", bufs=2,
                                  name=f"vch_{tcn}")
                    nc.scalar.copy(vch[:], acc[5][:])
                    tp4 = psA.tile([P, TCH], BF16, tag="tp", bufs=1,
                                   name=f"tp_{tcn}")
                    for j in range(4):
                        nc.tensor.transpose(
                            tp4[:, j * P:(j + 1) * P],
                            vch[:, j * P:(j + 1) * P], ident[:])
                    nc.vector.tensor_copy(
                        Vtok[b][:, cb * TCH:(cb + 1) * TCH], tp4[:])
                    cs = cos_sb[:, cb * TCH:(cb + 1) * TCH]
                    sn = sin_sb[:, cb * TCH:(cb + 1) * TCH]
                    morder = list(range(5))
                    slos, shis = {}, {}
                    for i, m in enumerate(morder):
                        slo = rp.tile([64, TCH], BF16, tag="slo", bufs=5,
                                      name=f"slo{m}_{tcn}")
                        shi = rp.tile([64, TCH], BF16, tag="shi", bufs=5,
                                      name=f"shi{m}_{tcn}")
                        if i % 2 == 0:
                            nc.scalar.copy(slo[:], acc[m][0:64, :])
                            nc.vector.tensor_copy(shi[:], acc[m][64:P, :])
                        else:
                            nc.vector.tensor_copy(slo[:], acc[m][0:64, :])
                            nc.scalar.copy(shi[:], acc[m][64:P, :])
                        slos[m] = slo
                        shis[m] = shi
                    def rope_muls(b=b, cb=cb, tcn=tcn, slos=slos,
                                  shis=shis, cs=cs, sn=sn):
                      for m in range(5):
                        slo, shi = slos[m], shis[m]
                        dst = qt_sb[m][b] if m < QH else KT_sb[b]
                        o_lo = dst[0:64, cb * TCH:(cb + 1) * TCH]
                        o_hi = dst[64:P, cb * TCH:(cb + 1) * TCH]
                        tA = rp.tile([64, TCH], BF16, tag="tA", bufs=1,
                                     name=f"tA{m}_{tcn}")
                        tB = rp.tile([64, TCH], BF16, tag="tB", bufs=1,
                                     name=f"tB{m}_{tcn}")
                        nc.vector.tensor_mul(tA[:], slo[:], cs)
                        nc.vector.tensor_mul(tB[:], shi[:], sn)
                        nc.vector.tensor_sub(o_lo, tA[:], tB[:])
                        tC = rp.tile([64, TCH], BF16, tag="tC", bufs=1,
                                     name=f"tC{m}_{tcn}")
                        tD = rp.tile([64, TCH], BF16, tag="tD", bufs=1,
                                     name=f"tD{m}_{tcn}")
                        nc.vector.tensor_mul(tC[:], slo[:], sn)
                        nc.vector.tensor_mul(tD[:], shi[:], cs)
                        nc.vector.tensor_add(o_hi, tC[:], tD[:])
                    if tcn == 0:
                        deferred_rope.append(rope_muls)
                    else:
                        rope_muls()

            # ------------- Phase B: attention + wo, per (batch, chunk) ------
            # PSUM creation order: sc(3 banks), pp(3), o(2) - lands on the
            # phase-A banks in the order they are drained.
            with (
                tc.tile_pool(name="psB", bufs=1, space="PSUM") as psB,
            ):
                # ---- tcn6: last QKV chunk runs with accumulators on
                # phase-B PSUM tags, so attention's first tiles wait on
                # per-bank drains instead of a whole-pool barrier ----
                t6 = NCH - 2
                b6, cb6 = divmod(t6, NCH // B)
                acc6 = [psB.tile([P, TCH], F32, tag="sc", bufs=3,
                                 name=f"a6_{m}") for m in range(3)]
                acc6 += [psB.tile([P, TCH], F32, tag="pp", bufs=3,
                                  name=f"a6_{m}") for m in range(3, 6)]
                for k in range(NKT):
                    if k % 4 == 0 and k <= NKT - 8:
                        load_xt(t6, k + 4, 4)
                    if k == 28:
                        load_xt(t6 + 1, 0, 4)
                    if k == 30:
                        load_xt(t6 + 1, 4, 4)
                    for m in range(6):
                        nc.tensor.matmul(
                            acc6[m][:], w_sb[m][:, k * HD:(k + 1) * HD],
                            xt_map[(t6, k)],
                            start=(k == 0), stop=(k == NKT - 1))
                vch6 = rp.tile([P, TCH], BF16, tag="vch", bufs=2,
                               name="vch_6")
                nc.scalar.copy(vch6[:], acc6[5][:])
                tp6 = psB.tile([P, TCH], BF16, tag="a7", bufs=1, name="tp_6")
                for jj in range(4):
                    nc.tensor.transpose(tp6[:, jj * P:(jj + 1) * P],
                                        vch6[:, jj * P:(jj + 1) * P],
                                        ident[:])
                nc.vector.tensor_copy(
                    Vtok[b6][:, cb6 * TCH:(cb6 + 1) * TCH], tp6[:])
                qk6 = {}
                for m in range(5):
                    slo = rp.tile([64, TCH], BF16, tag="slo", bufs=5,
                                  name=f"slo{m}_6")
                    shi = rp.tile([64, TCH], BF16, tag="shi", bufs=5,
                                  name=f"shi{m}_6")
                    if m % 2 == 0:
                        nc.scalar.copy(slo[:], acc6[m][0:64, :])
                        nc.vector.tensor_copy(shi[:], acc6[m][64:P, :])
                    else:
                        nc.vector.tensor_copy(slo[:], acc6[m][0:64, :])
                        nc.scalar.copy(shi[:], acc6[m][64:P, :])
                    qk6[m] = (slo, shi)

                def rope6():
                    cs = cos_sb[:, cb6 * TCH:(cb6 + 1) * TCH]
                    sn = sin_sb[:, cb6 * TCH:(cb6 + 1) * TCH]
                    for m in range(5):
                        slo, shi = qk6[m]
                        dst = qt_sb[m][b6] if m < QH else KT_sb[b6]
                        o_lo = dst[0:64, cb6 * TCH:(cb6 + 1) * TCH]
                        o_hi = dst[64:P, cb6 * TCH:(cb6 + 1) * TCH]
                        tA = rp.tile([64, TCH], BF16, tag="tA", bufs=1,
                                     name=f"tA{m}_6")
                        tB = rp.tile([64, TCH], BF16, tag="tB", bufs=1,
                                     name=f"tB{m}_6")
                        nc.vector.tensor_mul(tA[:], slo[:], cs)
                        nc.vector.tensor_mul(tB[:], shi[:], sn)
                        nc.vector.tensor_sub(o_lo, tA[:], tB[:])
                        tC = rp.tile([64, TCH], BF16, tag="tC", bufs=1,
                                     name=f"tC{m}_6")
                        tD = rp.tile([64, TCH], BF16, tag="tD", bufs=1,
                                     name=f"tD{m}_6")
                        nc.vector.tensor_mul(tC[:], slo[:], sn)
                        nc.vector.tensor_mul(tD[:], shi[:], cs)
                        nc.vector.tensor_add(o_hi, tC[:], tD[:])
                deferred_rope.append(rope6)

                pending = []
                mpend = []
                a7_state = {}

                def make_mpass(m, j):
                    # deferred tcn7 QKV: weight block m over k-tiles
                    # 8j..8j+7 into one PSUM bank (group spans units)
                    def emit():
                        tcn = NCH - 1
                        cb = 3
                        if j == 0:
                            a7_state[m] = psB.tile(
                                [P, TCH], F32, tag="a7", bufs=1,
                                name=f"a7_{m}")
                        acc7 = a7_state[m]
                        for kk in range(8 * j, 8 * j + 8):
                            if (tcn, kk) not in xt_map:
                                load_xt(tcn, kk - kk % 4, 4)
                            nc.tensor.matmul(
                                acc7[:], w_sb[m][:, kk * HD:(kk + 1) * HD],
                                xt_map[(tcn, kk)],
                                start=(kk == 0), stop=(kk == NKT - 1))
                        if j < 3:
                            return
                        for kk in range(NKT):
                            xt_map.pop((tcn, kk), None)
                        cs = cos_sb[:, cb * TCH:(cb + 1) * TCH]
                        sn = sin_sb[:, cb * TCH:(cb + 1) * TCH]
                        if m == 5:
                            vch = rp.tile([P, TCH], BF16, tag="vch", bufs=2,
                                          name="vch_7")
                            nc.scalar.copy(vch[:], acc7[:])
                            tp4 = psB.tile([P, TCH], BF16, tag="a7", bufs=1,
                                           name="tp_7")
                            for jj in range(4):
                                nc.tensor.transpose(
                                    tp4[:, jj * P:(jj + 1) * P],
                                    vch[:, jj * P:(jj + 1) * P], ident[:])
                            nc.vector.tensor_copy(
                                Vtok[1][:, cb * TCH:(cb + 1) * TCH], tp4[:])
                            return
                        slo = rp.tile([64, TCH], BF16, tag="slo", bufs=5,
                                      name=f"slo{m}_7")
                        shi = rp.tile([64, TCH], BF16, tag="shi", bufs=5,
                                      name=f"shi{m}_7")
                        if m % 2 == 0:
                            nc.scalar.copy(slo[:], acc7[0:64, :])
                            nc.vector.tensor_copy(shi[:], acc7[64:P, :])
                        else:
                            nc.vector.tensor_copy(slo[:], acc7[0:64, :])
                            nc.scalar.copy(shi[:], acc7[64:P, :])
                        dst = qt_sb[m][1] if m < QH else KT_sb[1]
                        o_lo = dst[0:64, cb * TCH:(cb + 1) * TCH]
                        o_hi = dst[64:P, cb * TCH:(cb + 1) * TCH]
                        tA = rp.tile([64, TCH], BF16, tag="tA", bufs=1,
                                     name=f"tA{m}_7")
                        tB = rp.tile([64, TCH], BF16, tag="tB", bufs=1,
                                     name=f"tB{m}_7")
                        nc.vector.tensor_mul(tA[:], slo[:], cs)
                        nc.vector.tensor_mul(tB[:], shi[:], sn)
                        nc.vector.tensor_sub(o_lo, tA[:], tB[:])
                        tC = rp.tile([64, TCH], BF16, tag="tC", bufs=1,
                                     name=f"tC{m}_7")
                        tD = rp.tile([64, TCH], BF16, tag="tD", bufs=1,
                                     name=f"tD{m}_7")
                        nc.vector.tensor_mul(tC[:], slo[:], sn)
                        nc.vector.tensor_mul(tD[:], shi[:], cs)
                        nc.vector.tensor_add(o_hi, tC[:], tD[:])
                    return emit

                for m_ in (5, 4, 0, 1, 2, 3):
                    for j_ in range(4):
                        mpend.append(make_mpass(m_, j_))

                def make_unit(b, sc, o_g, tj, half, njs):
                    def emit():
                        tt = 4 * sc + tj
                        w0 = njs[0] * TCH
                        w1 = (njs[-1] + 1) * TCH
                        tag = "ob" if len(njs) > 1 else "obf"
                        ob = obp.tile([P, w1 - w0], BF16, tag=tag,
                                      bufs=2 if tag == "ob" else 2,
                                      name=f"ob_{b}_{tt}_{half}")
                        for i, nn in enumerate(njs):
                            pp = psB.tile([P, TCH], F32, tag="pp", bufs=3,
                                          name=f"pp_{b}_{tt}_{nn}")
                            for kk in range(4):
                                nc.tensor.matmul(
                                    pp[:],
                                    o_g[kk][:, tj * P:(tj + 1) * P],
                                    wo_sb[kk][:, nn * TCH:(nn + 1) * TCH],
                                    start=(kk == 0), stop=(kk == 3))
                            dst = ob[:, i * TCH:(i + 1) * TCH]
                            if nn % 2 == 0:
                                nc.scalar.copy(dst, pp[:])
                            else:
                                nc.vector.tensor_copy(dst, pp[:])
                        nc.sync.dma_start(
                            part[b * S + tt * P:b * S + (tt + 1) * P, w0:w1],
                            ob[:])
                    return emit

                for b in range(B):
                    for sc in (0, 1, 2, 3):
                        nod = 4 * sc if variant == "causal" else 16
                        o_g = [obp.tile([P, TCH], BF16, tag=f"og{h}",
                                        bufs=3, name=f"og_{b}_{sc}_{h}")
                               for h in range(QH)]
                        for h in range(QH):
                            if mpend:
                                mpend.pop(0)()
                            for _ in range(2):
                                if len(pending) > 3:
                                    pending.pop(0)()
                            o_ps = psB.tile([P, TCH], F32, tag="o", bufs=1,
                                            name=f"o_{b}_{sc}_{h}")
                            E_ab = [ebp.tile([P, TCH], BF16, tag=f"ea{par}",
                                             bufs=2,
                                             name=f"ea{par}_{b}_{sc}_{h}")
                                    for par in range(2)]
                            qts = qt_sb[h][b][:, sc * TCH:(sc + 1) * TCH]
                            if variant == "causal":
                                # diagonal 512x512 block first: strip d covers
                                # query cols d*128..512, so the d=0 strip
                                # opens the o_ps accumulation group full-width
                                for d in range(4):
                                    w = (4 - d) * P
                                    c0 = d * P
                                    scd = psB.tile(
                                        [P, TCH], F32, tag="sc", bufs=3,
                                        name=f"sd_{b}_{sc}_{h}_{d}")
                                    nc.tensor.matmul(
                                        scd[:, 0:w],
                                        KT_sb[b][:,
                                                 (nod + d) * P:
                                                 (nod + d + 1) * P],
                                        qts[:, c0:TCH],
                                        start=True, stop=True)
                                    etd = ebp.tile(
                                        [P, TCH], BF16, tag="etd", bufs=3,
                                        name=f"ed_{b}_{sc}_{h}_{d}")
                                    nc.scalar.activation(etd[:, 0:w],
                                                         scd[:, 0:w], AF.Exp)
                                    nc.vector.tensor_mul(
                                        etd[:, 0:P], etd[:, 0:P], tri_sb[:])
                                    ea = E_ab[d % 2]
                                    if d < 2:
                                        nc.vector.tensor_copy(
                                            ea[:, c0:TCH], etd[:, 0:w])
                                    else:
                                        nc.vector.tensor_add(
                                            ea[:, c0:TCH], ea[:, c0:TCH],
                                            etd[:, 0:w])
                                    nc.tensor.matmul(
                                        o_ps[:, c0:TCH],
                                        Vtok[b][:, (nod + d) * P:
                                                (nod + d + 1) * P],
                                        etd[:, 0:w],
                                        start=(d == 0),
                                        stop=(nod == 0 and d == 3))
                            for tt in range(nod):
                                sc_ps = psB.tile([P, TCH], F32, tag="sc",
                                                 bufs=3,
                                                 name=f"s_{b}_{sc}_{h}_{tt}")
                                nc.tensor.matmul(
                                    sc_ps[:],
                                    KT_sb[b][:, tt * P:(tt + 1) * P],
                                    qts, start=True, stop=True)
                                et = ebp.tile([P, TCH], BF16, tag="et",
                                              bufs=5,
                                              name=f"et_{b}_{sc}_{h}_{tt}")
                                if variant == "general":
                                    etm = ebp.tile(
                                        [P, TCH], BF16, tag="etm", bufs=3,
                                        name=f"em_{b}_{sc}_{h}_{tt}")
                                    nc.scalar.activation(etm[:], sc_ps[:],
                                                         AF.Exp)
                                    mg = mkp.tile(
                                        [P, TCH], BF16, tag="mg", bufs=3,
                                        name=f"mg_{b}_{sc}_{h}_{tt}")
                                    nc.sync.dma_start(
                                        mg[:],
                                        emask[tt * P:(tt + 1) * P,
                                              sc * TCH:(sc + 1) * TCH])
                                    nc.vector.tensor_mul(et[:], etm[:], mg[:])
                                else:
                                    nc.scalar.activation(et[:], sc_ps[:],
                                                         AF.Exp)
                                if variant == "causal":
                                    ea = E_ab[tt % 2]
                                    if tt == 1:
                                        # ea1 cols 0:128 not covered by the
                                        # d=1 diagonal strip copy
                                        nc.vector.tensor_copy(
                                            ea[:, 0:P], et[:, 0:P])
                                        nc.vector.tensor_add(
                                            ea[:, P:TCH], ea[:, P:TCH],
                                            et[:, P:TCH])
                                    else:
                                        nc.vector.tensor_add(ea[:], ea[:],
                                                             et[:])
                                else:
                                    ea = E_ab[tt % 2]
                                    if tt < 2:
                                        nc.vector.tensor_copy(ea[:], et[:])
                                    else:
                                        nc.vector.tensor_add(ea[:], ea[:],
                                                             et[:])
                                nc.tensor.matmul(
                                    o_ps[:], Vtok[b][:, tt * P:(tt + 1) * P],
                                    et[:], start=(variant != "causal"
                                                  and tt == 0),
                                    stop=(tt == nod - 1))
                            ost = obp.tile([P, TCH], BF16, tag="ost",
                                           bufs=2, name=f"ost_{b}_{sc}_{h}")
                            if h % 2 == 0:
                                nc.scalar.copy(ost[:], o_ps[:])
                            else:
                                nc.vector.tensor_copy(ost[:], o_ps[:])
                            e_sum = ebp.tile([P, TCH], BF16, tag="es",
                                             bufs=1, name=f"es_{b}_{sc}_{h}")
                            if variant == "causal" and nod == 0:
                                nc.vector.tensor_copy(e_sum[:, 0:P],
                                                      E_ab[0][:, 0:P])
                                nc.vector.tensor_add(e_sum[:, P:TCH],
                                                     E_ab[0][:, P:TCH],
                                                     E_ab[1][:, P:TCH])
                            else:
                                nc.vector.tensor_add(e_sum[:], E_ab[0][:],
                                                     E_ab[1][:])
                            srec = obp.tile([P, TCH], F32, tag="sr", bufs=1,
                                            name=f"sr_{b}_{sc}_{h}")
                            nc.gpsimd.partition_all_reduce(
                                srec[:], e_sum[:], P, bass_isa.ReduceOp.add)
                            rec = obp.tile([P, TCH], BF16, tag="rec", bufs=1,
                                           name=f"rec_{b}_{sc}_{h}")
                            with nc.allow_low_precision(
                                    reason="softmax denom, 2e-2 gate"):
                                nc.vector.reciprocal(rec[:], srec[:])
                            nc.vector.tensor_mul(o_g[h][:], ost[:], rec[:])

                        if b == 0 and sc == 0:
                            for fn_ in deferred_rope:
                                fn_()
                            deferred_rope = []
                        last = (b == B - 1 and sc == 3)
                        for tj in range(4):
                            if last and tj == 3:
                                for u, njs in enumerate(
                                        ([0, 1], [2, 3], [4, 5], [6], [7])):
                                    pending.append(
                                        make_unit(b, sc, o_g, tj, u, njs))
                            else:
                                for half in range(2):
                                    pending.append(make_unit(
                                        b, sc, o_g, tj, half,
                                        [4 * half + i for i in range(4)]))
                for fn_ in pending:
                    fn_()
                pending = []

                xtp_cm.__exit__(None, None, None)
                rp_cm.__exit__(None, None, None)
                obp_cm.__exit__(None, None, None)
                mkp_cm.__exit__(None, None, None)
                ebp_cm.__exit__(None, None, None)

    nc.compile()
    return nc


def _get_prog(variant):
    if variant not in _prog_cache:
        _prog_cache[variant] = _build(variant)
    return _prog_cache[variant]


def prepare(inputs):
    """Host-side sharding prep: returns (variant, program, per-core input maps)."""
    x = np.asarray(inputs["x"], dtype=np.float32)
    wq = np.asarray(inputs["wq"], dtype=np.float32)
    wk = np.asarray(inputs["wk"], dtype=np.float32)
    wv = np.asarray(inputs["wv"], dtype=np.float32)
    wo = np.asarray(inputs["wo"], dtype=np.float32)
    fc = np.asarray(inputs["freqs_cos"], dtype=np.float32)
    fs = np.asarray(inputs["freqs_sin"], dtype=np.float32)
    mask = np.asarray(inputs["mask"], dtype=np.float32)

    xx = x.reshape(BS, DIM)
    # packed chunk-major: xT[p, tcn*(NKT*TCH) + k*TCH + t] = x[tcn*TCH+t, k*P+p]
    xT = np.ascontiguousarray(
        xx.reshape(NCH, TCH, NKT, P).transpose(3, 0, 2, 1)
        .reshape(P, NCH * NKT * TCH)).astype(NPBF)
    warm_x = np.ascontiguousarray(xx[0:TCH, 0:P].T).astype(NPBF)
    perm = np.concatenate([np.arange(0, HD, 2), np.arange(1, HD, 2)])
    wq_p = (wq.reshape(DIM, NH, HD)[:, :, perm] / math.sqrt(HD))
    wk_p = wk.reshape(DIM, NKV, HD)[:, :, perm]
    cosT = np.ascontiguousarray(fc.T).astype(NPBF)
    sinT = np.ascontiguousarray(fs.T).astype(NPBF)

    if not mask.any():
        variant = "none"
    else:
        il, jl = np.tril_indices(S)
        iu, ju = np.triu_indices(S, 1)
        if np.all(mask[il, jl] == 0.0) and np.all(mask[iu, ju] <= -1e8):
            variant = "causal"
        else:
            variant = "general"

    tri128 = None
    emaskT = None
    if variant == "causal":
        t = np.arange(P)[:, None]
        q = np.arange(P)[None, :]
        tri128 = (q >= t).astype(NPBF)
    elif variant == "general":
        with np.errstate(under="ignore", over="ignore"):
            emaskT = np.ascontiguousarray(np.exp(mask).T).astype(NPBF)

    nc = _get_prog(variant)

    in_maps = []
    for c in range(NCORES):
        wqc = wq_p[:, c * QH:(c + 1) * QH, :]                    # [DIM,QH,HD]
        wqc = np.ascontiguousarray(
            wqc.reshape(NKT, P, QH, HD).transpose(1, 2, 0, 3)
            .reshape(P, QH * NKT * HD)).astype(NPBF)
        wkc = np.ascontiguousarray(
            wk_p[:, c, :].reshape(NKT, P, HD).transpose(1, 0, 2)
            .reshape(P, NKT * HD)).astype(NPBF)
        wvc = np.ascontiguousarray(
            wv[:, c * HD:(c + 1) * HD].reshape(NKT, P, HD).transpose(1, 0, 2)
            .reshape(P, NKT * HD)).astype(NPBF)
        warm = np.concatenate(
            [warm_x]
            + [wqc[:, m * NKT * HD:m * NKT * HD + HD] for m in range(QH)]
            + [wkc[:, 0:HD], wvc[:, 0:HD]], axis=1)
        m = {
            "xT": xT,
            "warm": np.ascontiguousarray(warm).astype(NPBF),
            "wq": wqc,
            "wk": wkc,
            "wv": wvc,
            "wo": np.ascontiguousarray(
                wo[c * DQ:(c + 1) * DQ, :]).astype(NPBF),
            "cosT": cosT,
            "sinT": sinT,
        }
        if variant == "causal":
            m["tri"] = tri128
        elif variant == "general":
            m["emaskT"] = emaskT
        in_maps.append(m)
    return variant, nc, in_maps


def kernel(**inputs):
    global LAST_RESULTS
    variant, nc, in_maps = prepare(inputs)
    out = None
    for attempt in range(3):
        res = run_bass_kernel_spmd(nc, in_maps, core_ids=list(range(NCORES)))
        LAST_RESULTS = res
        out = np.zeros((BS, DIM), dtype=np.float64)
        ok = True
        for c in range(NCORES):
            p = np.asarray(res.results[c]["part"], dtype=np.float64)
            # flaky-execution guard: a healthy partial is finite, O(1)-scale,
            # and every token row is nonzero (dense projection of dense
            # data); huge values, NaNs, or any all-zero row mean the device
            # produced a bad/partial result -> re-execute
            rowmax = np.abs(p).max(axis=1)
            if not np.isfinite(p).all() or p.max() > 1e3 \
                    or p.min() < -1e3 or rowmax.min() == 0.0:
                ok = False
            out += p
        if ok:
            break
    return out.reshape(B, S, DIM).astype(np.float32)


# revision 75
# speedup vs baseline: 1.0017x; 1.0016x over previous
"""Trainium2 Bass kernel for GQA attention with RoPE (dense transformer block).

Strategy (8-way tensor parallel over heads, per the sharding hint):
  - Each of the 8 NeuronCores gets 4 Q heads + 1 KV head (KV head c, Q heads
    4c..4c+3); host sums the 8 partial bf16 outputs (the "all-reduce after wo").
  - All matmul operands are bf16 (fp32 PSUM accumulation). Host pre-packs
    weights so every DMA is wide and contiguous.
  - Causal attention computes the exact block triangle: the diagonal 512x512
    block runs first as one row-strip per KV tile covering only the
    surviving query columns, so each o_ps PSUM bank sees a single
    accumulation group; off-diagonal KV tiles then accumulate at full
    512-query width. Softmax is unnormalized exp; denominators come from
    two alternating elementwise accumulators (DVE) + one GPSIMD
    partition_all_reduce per (batch, head, chunk). o_ps is staged to SBUF
    right after its accumulation stops (alternating ACT/DVE) so its single
    PSUM bank recycles long before the normalize chain completes.
  - The LAST two QKV chunks leave phase A: the second-to-last runs on
    phase B's own PSUM tags (so attention's first tiles wait on per-bank
    drains, not a whole-pool barrier) and the last is pre-seeded as 24
    single-bank filler units (one weight block over 8 k-tiles each,
    accumulation group spanning units) popped one per head through early
    phase B - ready PE work exactly where the attention pipeline is
    otherwise filler-starved; their RoPE is deferred into phase B.
  - x is packed chunk-major in DRAM so one DMA loads 4 k-tiles of a chunk;
    a packed "warm" DRAM tensor (first x tile + k=0 weight slices) gates
    the first matmul on a single DMA while dummy matmuls burn the PE
    p-state ramp; weight/cos/sin/wo loads are staged just-in-time around
    the xt stream, wo loads gated against sequencer hoisting; chunk 0's
    RoPE is deferred until its cos/sin tables land during chunk 1.
  - The wo projection is split into half-token-tile units emitted with a
    delay so a deep ready-pool of wo matmuls fills the PE gaps left by the
    scores->exp->PV dependency chain; the final token tile flushes in
    smaller units so the last PSUM->SBUF->DRAM drain is short.
"""
import math

import numpy as np
import ml_dtypes

import concourse.bass as bass
import concourse.tile as tile
from concourse import bacc, bass_isa, mybir
from concourse.bass_utils import run_bass_kernel_spmd
from concourse.masks import make_identity

B, S, DIM = 2, 2048, 4096
NH, NKV, HD = 32, 8, 128
BS = B * S
NCORES = 8
QH = NH // NCORES          # 4 Q heads per core
DQ = QH * HD               # 512
TCH = 512                  # token chunk
NCH = BS // TCH            # 8 chunks
NKT = DIM // 128           # 32 contraction tiles
P = 128

F32 = mybir.dt.float32
BF16 = mybir.dt.bfloat16
AF = mybir.ActivationFunctionType
NPBF = ml_dtypes.bfloat16

_prog_cache = {}
LAST_RESULTS = None


def _build(variant):
    """variant: 'causal' | 'none' | 'general'"""
    nc = bacc.Bacc(None, target_bir_lowering=False)
    # xTp packed chunk-major: col = tcn*(NKT*TCH) + k*TCH + t, so one DMA
    # loads several consecutive k-tiles of the same token chunk
    xT = nc.dram_tensor("xT", [P, NCH * NKT * TCH], BF16,
                        kind="ExternalInput")
    warm = nc.dram_tensor("warm", [P, TCH + 6 * HD], BF16,
                          kind="ExternalInput")
    wq = nc.dram_tensor("wq", [P, QH * NKT * HD], BF16, kind="ExternalInput")
    wk = nc.dram_tensor("wk", [P, NKT * HD], BF16, kind="ExternalInput")
    wv = nc.dram_tensor("wv", [P, NKT * HD], BF16, kind="ExternalInput")
    wo = nc.dram_tensor("wo", [DQ, DIM], BF16, kind="ExternalInput")
    cosT = nc.dram_tensor("cosT", [64, S], BF16, kind="ExternalInput")
    sinT = nc.dram_tensor("sinT", [64, S], BF16, kind="ExternalInput")
    tri = None
    emask = None
    if variant == "causal":
        tri = nc.dram_tensor("tri", [P, P], BF16, kind="ExternalInput")
    elif variant == "general":
        emask = nc.dram_tensor("emaskT", [S, S], BF16, kind="ExternalInput")
    part = nc.dram_tensor("part", [BS, DIM], BF16, kind="ExternalOutput")

    with tile.TileContext(nc) as tc:
        with (
            tc.tile_pool(name="const", bufs=1) as constp,
            tc.tile_pool(name="big", bufs=1) as bigp,
        ):
            warm_sb = constp.tile([P, TCH + 6 * HD], BF16)
            ident = constp.tile([P, P], BF16)
            make_identity(nc, ident)
            cos_sb = constp.tile([64, S], BF16)
            sin_sb = constp.tile([64, S], BF16)
            tri_sb = None
            if variant == "causal":
                tri_sb = constp.tile([P, P], BF16)

            # persistent per-batch activations (bf16)
            KT_sb = [bigp.tile([P, S], BF16, name=f"KT{b}") for b in range(B)]
            Vtok = [bigp.tile([P, S], BF16, name=f"Vtok{b}") for b in range(B)]
            qt_sb = [[bigp.tile([P, S], BF16, name=f"qt{h}_{b}")
                      for b in range(B)] for h in range(QH)]
            w_sb = [bigp.tile([P, NKT * HD], BF16, name=f"w{m}")
                    for m in range(6)]
            wo_sb = [bigp.tile([P, DIM], BF16, name=f"wo{kk}")
                     for kk in range(4)]

            wsrc = [wq[:, m * NKT * HD:(m + 1) * NKT * HD] for m in range(QH)]
            wsrc += [wk[:, :], wv[:, :]]

            # B-phase SBUF pools open FIRST so their tiles never alias the
            # phase-A scratch (whose release depends on the tcn7 rope tail)
            ebp_cm = tc.tile_pool(name="ebp", bufs=1)
            ebp = ebp_cm.__enter__()
            mkp_cm = tc.tile_pool(name="mkp", bufs=1)
            mkp = mkp_cm.__enter__()
            obp_cm = tc.tile_pool(name="obp", bufs=1)
            obp = obp_cm.__enter__()

            deferred_rope = []
            # rp stays open through phase B (tcn7 rope muls are deferred
            # there so the transition DVE queue is clear for attention)
            rp_cm = tc.tile_pool(name="rp", bufs=1)
            rp = rp_cm.__enter__()
            xtp_cm = tc.tile_pool(name="xtp", bufs=1)
            xtp = xtp_cm.__enter__()
            # ---------------- Phase A: QKV projection + RoPE ----------------
            with (
                tc.tile_pool(name="psA", bufs=1, space="PSUM") as psA,
            ):
                nc.sync.dma_start(warm_sb[:], warm[:, :])
                # p-state spin-up: burn the PE clock ramp on dummy matmuls
                # while the first DMA is in flight
                spin = ebp.tile([P, TCH], BF16, tag="et", bufs=5,
                                name="spin")
                nc.vector.memset(spin[:], 0.0)
                sps = psA.tile([P, TCH], F32, tag="acc", bufs=7,
                               name="spin_ps")
                for _ in range(6):
                    nc.tensor.matmul(sps[:], spin[:, 0:P], spin[:],
                                     start=True, stop=True)
                xt_map = {}

                def load_xt(t_, k0, nk):
                    t = xtp.tile([P, nk * TCH], BF16, tag="xt",
                                 bufs=3 if variant == "causal" else 2,
                                 name=f"xt_{t_}_{k0}")
                    nc.sync.dma_start(
                        t[:], xT[:, t_ * NKT * TCH + k0 * TCH:
                                 t_ * NKT * TCH + (k0 + nk) * TCH])
                    for kk_ in range(k0, k0 + nk):
                        xt_map[(t_, kk_)] = t[:, (kk_ - k0) * TCH:
                                              (kk_ - k0 + 1) * TCH]

                for tcn in range(NCH - 2):
                    b, cb = divmod(tcn, NCH // B)
                    acc = [psA.tile([P, TCH], F32, tag="acc", bufs=7,
                                    name=f"acc{m}_{tcn}") for m in range(6)]
                    if tcn == 0:
                        load_xt(0, 1, 3)
                    for k in range(NKT):
                        if tcn == 0 and k == 0:
                            xsrc = warm_sb[:, 0:TCH]
                            wslc = [warm_sb[:, TCH + m * HD:TCH + (m + 1) * HD]
                                    for m in range(6)]
                        else:
                            xsrc = xt_map[(tcn, k)]
                            wslc = [w_sb[m][:, k * HD:(k + 1) * HD]
                                    for m in range(6)]
                        # xt prefetch: next 4-k tile; next chunk's first
                        # tile is requested from inside this chunk's tail
                        if k % 4 == 0 and k <= NKT - 8:
                            load_xt(tcn, k + 4, 4)
                        if k == 28 and tcn < NCH - 1:
                            load_xt(tcn + 1, 0, 4)
                        # staged just-in-time weight loads; cos/sin/tri
                        # land late in chunk 0 (needed only at its drain)
                        wstage = {1: (0, 4), 2: (4, 10), 3: (10, 16),
                                  5: (16, 24), 10: (24, 32)}
                        if tcn == 0 and k in wstage:
                            lo, hi = wstage[k]
                            for m in range(6):
                                eng = nc.gpsimd if m % 2 == 0 else nc.sync
                                eng.dma_start(w_sb[m][:, lo * HD:hi * HD],
                                              wsrc[m][:, lo * HD:hi * HD])
                        if tcn == 1 and k in (0, 1):
                            src = (cos_sb, cosT) if k == 0 else (sin_sb, sinT)
                            nc.gpsimd.dma_start(src[0][:, :], src[1][:, :])
                        if tcn == 1 and k == 2 and variant == "causal":
                            nc.gpsimd.dma_start(tri_sb[:], tri[:, :])
                        if tcn == 1 and k == 3 and deferred_rope:
                            deferred_rope.pop(0)()
                        if tcn == 2 and k == 0:
                            # dispatch gate: Pool SEQ would otherwise hoist
                            # the dep-free wo loads into chunk 0's DMA crunch
                            gate = rp.tile([P, 16], BF16, tag="gate", bufs=1,
                                           name="gate")
                            nc.gpsimd.tensor_copy(gate[:], KT_sb[0][:, 0:16])
                        if tcn in (2, 3) and k in (0, 8, 16, 24):
                            kk = (tcn - 2) * 2 + (k // 16)
                            hf = (k // 8) % 2
                            nc.gpsimd.dma_start(
                                wo_sb[kk][:, hf * 2048:(hf + 1) * 2048],
                                wo[kk * P:(kk + 1) * P,
                                   hf * 2048:(hf + 1) * 2048])

                        for m in range(6):
                            nc.tensor.matmul(
                                acc[m][:], wslc[m], xsrc,
                                start=(k == 0), stop=(k == NKT - 1))

                    # ---- drain: all copies first, rope muls after ----
                    vch = rp.tile([P, TCH], BF16, tag="vch", bufs=2,
                                  name=f"vch_{tcn}")
                    nc.scalar.copy(vch[:], acc[5][:])
                    tp4 = psA.tile([P, TCH], BF16, tag="tp", bufs=1,
                                   name=f"tp_{tcn}")
                    for j in range(4):
                        nc.tensor.transpose(
                            tp4[:, j * P:(j + 1) * P],
                            vch[:, j * P:(j + 1) * P], ident[:])
                    nc.vector.tensor_copy(
                        Vtok[b][:, cb * TCH:(cb + 1) * TCH], tp4[:])
                    cs = cos_sb[:, cb * TCH:(cb + 1) * TCH]
                    sn = sin_sb[:, cb * TCH:(cb + 1) * TCH]
                    morder = list(range(5))
                    slos, shis = {}, {}
                    for i, m in enumerate(morder):
                        slo = rp.tile([64, TCH], BF16, tag="slo", bufs=5,
                                      name=f"slo{m}_{tcn}")
                        shi = rp.tile([64, TCH], BF16, tag="shi", bufs=5,
                                      name=f"shi{m}_{tcn}")
                        if i % 2 == 0:
                            nc.scalar.copy(slo[:], acc[m][0:64, :])
                            nc.vector.tensor_copy(shi[:], acc[m][64:P, :])
                        else:
                            nc.vector.tensor_copy(slo[:], acc[m][0:64, :])
                            nc.scalar.copy(shi[:], acc[m][64:P, :])
                        slos[m] = slo
                        shis[m] = shi
                    def rope_muls(b=b, cb=cb, tcn=tcn, slos=slos,
                                  shis=shis, cs=cs, sn=sn):
                      for m in range(5):
                        slo, shi = slos[m], shis[m]
                        dst = qt_sb[m][b] if m < QH else KT_sb[b]
                        o_lo = dst[0:64, cb * TCH:(cb + 1) * TCH]
                        o_hi = dst[64:P, cb * TCH:(cb + 1) * TCH]
                        tA = rp.tile([64, TCH], BF16, tag="tA", bufs=1,
                                     name=f"tA{m}_{tcn}")
                        tB = rp.tile([64, TCH], BF16, tag="tB", bufs=1,
                                     name=f"tB{m}_{tcn}")
                        nc.vector.tensor_mul(tA[:], slo[:], cs)
                        nc.vector.tensor_mul(tB[:], shi[:], sn)
                        nc.vector.tensor_sub(o_lo, tA[:], tB[:])
                        tC = rp.tile([64, TCH], BF16, tag="tC", bufs=1,
                                     name=f"tC{m}_{tcn}")
                        tD = rp.tile([64, TCH], BF16, tag="tD", bufs=1,
                                     name=f"tD{m}_{tcn}")
                        nc.vector.tensor_mul(tC[:], slo[:], sn)
                        nc.vector.tensor_mul(tD[:], shi[:], cs)
                        nc.vector.tensor_add(o_hi, tC[:], tD[:])
                    if tcn == 0:
                        deferred_rope.append(rope_muls)
                    else:
                        rope_muls()

            # ------------- Phase B: attention + wo, per (batch, chunk) ------
            # PSUM creation order: sc(3 banks), pp(3), o(2) - lands on the
            # phase-A banks in the order they are drained.
            with (
                tc.tile_pool(name="psB", bufs=1, space="PSUM") as psB,
            ):
                # ---- tcn6: last QKV chunk runs with accumulators on
                # phase-B PSUM tags, so attention's first tiles wait on
                # per-bank drains instead of a whole-pool barrier ----
                t6 = NCH - 2
                b6, cb6 = divmod(t6, NCH // B)
                acc6 = [psB.tile([P, TCH], F32, tag="sc", bufs=3,
                                 name=f"a6_{m}") for m in range(3)]
                acc6 += [psB.tile([P, TCH], F32, tag="pp", bufs=3,
                                  name=f"a6_{m}") for m in range(3, 6)]
                for k in range(NKT):
                    if k % 4 == 0 and k <= NKT - 8:
                        load_xt(t6, k + 4, 4)
                    if k == 28:
                        load_xt(t6 + 1, 0, 4)
                    if k == 30:
                        load_xt(t6 + 1, 4, 4)
                    for m in range(6):
                        nc.tensor.matmul(
                            acc6[m][:], w_sb[m][:, k * HD:(k + 1) * HD],
                            xt_map[(t6, k)],
                            start=(k == 0), stop=(k == NKT - 1))
                vch6 = rp.tile([P, TCH], BF16, tag="vch", bufs=2,
                               name="vch_6")
                nc.scalar.copy(vch6[:], acc6[5][:])
                tp6 = psB.tile([P, TCH], BF16, tag="a7", bufs=1, name="tp_6")
                for jj in range(4):
                    nc.tensor.transpose(tp6[:, jj * P:(jj + 1) * P],
                                        vch6[:, jj * P:(jj + 1) * P],
                                        ident[:])
                nc.vector.tensor_copy(
                    Vtok[b6][:, cb6 * TCH:(cb6 + 1) * TCH], tp6[:])
                qk6 = {}
                for m in range(5):
                    slo = rp.tile([64, TCH], BF16, tag="slo", bufs=5,
                                  name=f"slo{m}_6")
                    shi = rp.tile([64, TCH], BF16, tag="shi", bufs=5,
                                  name=f"shi{m}_6")
                    if m % 2 == 0:
                        nc.scalar.copy(slo[:], acc6[m][0:64, :])
                        nc.vector.tensor_copy(shi[:], acc6[m][64:P, :])
                    else:
                        nc.vector.tensor_copy(slo[:], acc6[m][0:64, :])
                        nc.scalar.copy(shi[:], acc6[m][64:P, :])
                    qk6[m] = (slo, shi)

                def rope6():
                    cs = cos_sb[:, cb6 * TCH:(cb6 + 1) * TCH]
                    sn = sin_sb[:, cb6 * TCH:(cb6 + 1) * TCH]
                    for m in range(5):
                        slo, shi = qk6[m]
                        dst = qt_sb[m][b6] if m < QH else KT_sb[b6]
                        o_lo = dst[0:64, cb6 * TCH:(cb6 + 1) * TCH]
                        o_hi = dst[64:P, cb6 * TCH:(cb6 + 1) * TCH]
                        tA = rp.tile([64, TCH], BF16, tag="tA", bufs=1,
                                     name=f"tA{m}_6")
                        tB = rp.tile([64, TCH], BF16, tag="tB", bufs=1,
                                     name=f"tB{m}_6")
                        nc.vector.tensor_mul(tA[:], slo[:], cs)
                        nc.vector.tensor_mul(tB[:], shi[:], sn)
                        nc.vector.tensor_sub(o_lo, tA[:], tB[:])
                        tC = rp.tile([64, TCH], BF16, tag="tC", bufs=1,
                                     name=f"tC{m}_6")
                        tD = rp.tile([64, TCH], BF16, tag="tD", bufs=1,
                                     name=f"tD{m}_6")
                        nc.vector.tensor_mul(tC[:], slo[:], sn)
                        nc.vector.tensor_mul(tD[:], shi[:], cs)
                        nc.vector.tensor_add(o_hi, tC[:], tD[:])
                deferred_rope.append(rope6)

                pending = []
                mpend = []
                a7_state = {}

                def make_mpass(m, j):
                    # deferred tcn7 QKV: weight block m over k-tiles
                    # 8j..8j+7 into one PSUM bank (group spans units)
                    def emit():
                        tcn = NCH - 1
                        cb = 3
                        if j == 0:
                            a7_state[m] = psB.tile(
                                [P, TCH], F32, tag="a7", bufs=1,
                                name=f"a7_{m}")
                        acc7 = a7_state[m]
                        for kk in range(8 * j, 8 * j + 8):
                            if (tcn, kk) not in xt_map:
                                load_xt(tcn, kk - kk % 4, 4)
                            nc.tensor.matmul(
                                acc7[:], w_sb[m][:, kk * HD:(kk + 1) * HD],
                                xt_map[(tcn, kk)],
                                start=(kk == 0), stop=(kk == NKT - 1))
                        if j < 3:
                            return
                        for kk in range(NKT):
                            xt_map.pop((tcn, kk), None)
                        cs = cos_sb[:, cb * TCH:(cb + 1) * TCH]
                        sn = sin_sb[:, cb * TCH:(cb + 1) * TCH]
                        if m == 5:
                            vch = rp.tile([P, TCH], BF16, tag="vch", bufs=2,
                                          name="vch_7")
                            nc.scalar.copy(vch[:], acc7[:])
                            tp4 = psB.tile([P, TCH], BF16, tag="a7", bufs=1,
                                           name="tp_7")
                            for jj in range(4):
                                nc.tensor.transpose(
                                    tp4[:, jj * P:(jj + 1) * P],
                                    vch[:, jj * P:(jj + 1) * P], ident[:])
                            nc.vector.tensor_copy(
                                Vtok[1][:, cb * TCH:(cb + 1) * TCH], tp4[:])
                            return
                        slo = rp.tile([64, TCH], BF16, tag="slo", bufs=5,
                                      name=f"slo{m}_7")
                        shi = rp.tile([64, TCH], BF16, tag="shi", bufs=5,
                                      name=f"shi{m}_7")
                        if m % 2 == 0:
                            nc.scalar.copy(slo[:], acc7[0:64, :])
                            nc.vector.tensor_copy(shi[:], acc7[64:P, :])
                        else:
                            nc.vector.tensor_copy(slo[:], acc7[0:64, :])
                            nc.scalar.copy(shi[:], acc7[64:P, :])
                        dst = qt_sb[m][1] if m < QH else KT_sb[1]
                        o_lo = dst[0:64, cb * TCH:(cb + 1) * TCH]
                        o_hi = dst[64:P, cb * TCH:(cb + 1) * TCH]
                        tA = rp.tile([64, TCH], BF16, tag="tA", bufs=1,
                                     name=f"tA{m}_7")
                        tB = rp.tile([64, TCH], BF16, tag="tB", bufs=1,
                                     name=f"tB{m}_7")
                        nc.vector.tensor_mul(tA[:], slo[:], cs)
                        nc.vector.tensor_mul(tB[:], shi[:], sn)
                        nc.vector.tensor_sub(o_lo, tA[:], tB[:])
                        tC = rp.tile([64, TCH], BF16, tag="tC", bufs=1,
                                     name=f"tC{m}_7")
                        tD = rp.tile([64, TCH], BF16, tag="tD", bufs=1,
                                     name=f"tD{m}_7")
                        nc.vector.tensor_mul(tC[:], slo[:], sn)
                        nc.vector.tensor_mul(tD[:], shi[:], cs)
                        nc.vector.tensor_add(o_hi, tC[:], tD[:])
                    return emit

                for m_ in (5, 4, 0, 1, 2, 3):
                    for j_ in range(4):
                        mpend.append(make_mpass(m_, j_))

                def make_unit(b, sc, o_g, tj, half, njs):
                    def emit():
                        tt = 4 * sc + tj
                        w0 = njs[0] * TCH
                        w1 = (njs[-1] + 1) * TCH
                        tag = "ob" if len(njs) > 1 else "obf"
                        ob = obp.tile([P, w1 - w0], BF16, tag=tag,
                                      bufs=2 if tag == "ob" else 2,
                                      name=f"ob_{b}_{tt}_{half}")
                        for i, nn in enumerate(njs):
                            pp = psB.tile([P, TCH], F32, tag="pp", bufs=3,
                                          name=f"pp_{b}_{tt}_{nn}")
                            for kk in range(4):
                                nc.tensor.matmul(
                                    pp[:],
                                    o_g[kk][:, tj * P:(tj + 1) * P],
                                    wo_sb[kk][:, nn * TCH:(nn + 1) * TCH],
                                    start=(kk == 0), stop=(kk == 3))
                            dst = ob[:, i * TCH:(i + 1) * TCH]
                            if nn % 2 == 1:
                                nc.scalar.copy(dst, pp[:])
                            else:
                                nc.vector.tensor_copy(dst, pp[:])
                        nc.sync.dma_start(
                            part[b * S + tt * P:b * S + (tt + 1) * P, w0:w1],
                            ob[:])
                    return emit

                for b in range(B):
                    for sc in (0, 1, 2, 3):
                        nod = 4 * sc if variant == "causal" else 16
                        o_g = [obp.tile([P, TCH], BF16, tag=f"og{h}",
                                        bufs=3, name=f"og_{b}_{sc}_{h}")
                               for h in range(QH)]
                        for h in range(QH):
                            if mpend:
                                mpend.pop(0)()
                            for _ in range(2):
                                if len(pending) > 3:
                                    pending.pop(0)()
                            o_ps = psB.tile([P, TCH], F32, tag="o", bufs=1,
                                            name=f"o_{b}_{sc}_{h}")
                            E_ab = [ebp.tile([P, TCH], BF16, tag=f"ea{par}",
                                             bufs=2,
                                             name=f"ea{par}_{b}_{sc}_{h}")
                                    for par in range(2)]
                            qts = qt_sb[h][b][:, sc * TCH:(sc + 1) * TCH]
                            if variant == "causal":
                                # diagonal 512x512 block first: strip d covers
                                # query cols d*128..512, so the d=0 strip
                                # opens the o_ps accumulation group full-width
                                for d in range(4):
                                    w = (4 - d) * P
                                    c0 = d * P
                                    scd = psB.tile(
                                        [P, TCH], F32, tag="sc", bufs=3,
                                        name=f"sd_{b}_{sc}_{h}_{d}")
                                    nc.tensor.matmul(
                                        scd[:, 0:w],
                                        KT_sb[b][:,
                                                 (nod + d) * P:
                                                 (nod + d + 1) * P],
                                        qts[:, c0:TCH],
                                        start=True, stop=True)
                                    etd = ebp.tile(
                                        [P, TCH], BF16, tag="etd", bufs=3,
                                        name=f"ed_{b}_{sc}_{h}_{d}")
                                    nc.scalar.activation(etd[:, 0:w],
                                                         scd[:, 0:w], AF.Exp)
                                    nc.vector.tensor_mul(
                                        etd[:, 0:P], etd[:, 0:P], tri_sb[:])
                                    ea = E_ab[d % 2]
                                    if d < 2:
                                        nc.vector.tensor_copy(
                                            ea[:, c0:TCH], etd[:, 0:w])
                                    else:
                                        nc.vector.tensor_add(
                                            ea[:, c0:TCH], ea[:, c0:TCH],
                                            etd[:, 0:w])
                                    nc.tensor.matmul(
                                        o_ps[:, c0:TCH],
                                        Vtok[b][:, (nod + d) * P:
                                                (nod + d + 1) * P],
                                        etd[:, 0:w],
                                        start=(d == 0),
                                        stop=(nod == 0 and d == 3))
                            for tt in range(nod):
                                sc_ps = psB.tile([P, TCH], F32, tag="sc",
                                                 bufs=3,
                                                 name=f"s_{b}_{sc}_{h}_{tt}")
                                nc.tensor.matmul(
                                    sc_ps[:],
                                    KT_sb[b][:, tt * P:(tt + 1) * P],
                                    qts, start=True, stop=True)
                                et = ebp.tile([P, TCH], BF16, tag="et",
                                              bufs=5,
                                              name=f"et_{b}_{sc}_{h}_{tt}")
                                if variant == "general":
                                    etm = ebp.tile(
                                        [P, TCH], BF16, tag="etm", bufs=3,
                                        name=f"em_{b}_{sc}_{h}_{tt}")
                                    nc.scalar.activation(etm[:], sc_ps[:],
                                                         AF.Exp)
                                    mg = mkp.tile(
                                        [P, TCH], BF16, tag="mg", bufs=3,
                                        name=f"mg_{b}_{sc}_{h}_{tt}")
                                    nc.sync.dma_start(
                                        mg[:],
                                        emask[tt * P:(tt + 1) * P,
                                              sc * TCH:(sc + 1) * TCH])
                                    nc.vector.tensor_mul(et[:], etm[:], mg[:])
                                else:
                                    nc.scalar.activation(et[:], sc_ps[:],
                                                         AF.Exp)
                                if variant == "causal":
                                    ea = E_ab[tt % 2]
                                    if tt == 1:
                                        # ea1 cols 0:128 not covered by the
                                        # d=1 diagonal strip copy
                                        nc.vector.tensor_copy(
                                            ea[:, 0:P], et[:, 0:P])
                                        nc.vector.tensor_add(
                                            ea[:, P:TCH], ea[:, P:TCH],
                                            et[:, P:TCH])
                                    else:
                                        nc.vector.tensor_add(ea[:], ea[:],
                                                             et[:])
                                else:
                                    ea = E_ab[tt % 2]
                                    if tt < 2:
                                        nc.vector.tensor_copy(ea[:], et[:])
                                    else:
                                        nc.vector.tensor_add(ea[:], ea[:],
                                                             et[:])
                                nc.tensor.matmul(
                                    o_ps[:], Vtok[b][:, tt * P:(tt + 1) * P],
                                    et[:], start=(variant != "causal"
                                                  and tt == 0),
                                    stop=(tt == nod - 1))
                            ost = obp.tile([P, TCH], BF16, tag="ost",
                                           bufs=2, name=f"ost_{b}_{sc}_{h}")
                            if h % 2 == 0:
                                nc.scalar.copy(ost[:], o_ps[:])
                            else:
                                nc.vector.tensor_copy(ost[:], o_ps[:])
                            e_sum = ebp.tile([P, TCH], BF16, tag="es",
                                             bufs=1, name=f"es_{b}_{sc}_{h}")
                            if variant == "causal" and nod == 0:
                                nc.vector.tensor_copy(e_sum[:, 0:P],
                                                      E_ab[0][:, 0:P])
                                nc.vector.tensor_add(e_sum[:, P:TCH],
                                                     E_ab[0][:, P:TCH],
                                                     E_ab[1][:, P:TCH])
                            else:
                                nc.vector.tensor_add(e_sum[:], E_ab[0][:],
                                                     E_ab[1][:])
                            srec = obp.tile([P, TCH], F32, tag="sr", bufs=1,
                                            name=f"sr_{b}_{sc}_{h}")
                            nc.gpsimd.partition_all_reduce(
                                srec[:], e_sum[:], P, bass_isa.ReduceOp.add)
                            rec = obp.tile([P, TCH], BF16, tag="rec", bufs=1,
                                           name=f"rec_{b}_{sc}_{h}")
                            with nc.allow_low_precision(
                                    reason="softmax denom, 2e-2 gate"):
                                nc.vector.reciprocal(rec[:], srec[:])
                            nc.vector.tensor_mul(o_g[h][:], ost[:], rec[:])

                        if b == 0 and sc == 0:
                            for fn_ in deferred_rope:
                                fn_()
                            deferred_rope = []
                        last = (b == B - 1 and sc == 3)
                        for tj in range(4):
                            if last and tj == 3:
                                for u, njs in enumerate(
                                        ([0, 1], [2, 3], [4, 5], [6], [7])):
                                    pending.append(
                                        make_unit(b, sc, o_g, tj, u, njs))
                            else:
                                for half in range(2):
                                    pending.append(make_unit(
                                        b, sc, o_g, tj, half,
                                        [4 * half + i for i in range(4)]))
                for fn_ in pending:
                    fn_()
                pending = []

                xtp_cm.__exit__(None, None, None)
                rp_cm.__exit__(None, None, None)
                obp_cm.__exit__(None, None, None)
                mkp_cm.__exit__(None, None, None)
                ebp_cm.__exit__(None, None, None)

    nc.compile()
    return nc


def _get_prog(variant):
    if variant not in _prog_cache:
        _prog_cache[variant] = _build(variant)
    return _prog_cache[variant]


def prepare(inputs):
    """Host-side sharding prep: returns (variant, program, per-core input maps)."""
    x = np.asarray(inputs["x"], dtype=np.float32)
    wq = np.asarray(inputs["wq"], dtype=np.float32)
    wk = np.asarray(inputs["wk"], dtype=np.float32)
    wv = np.asarray(inputs["wv"], dtype=np.float32)
    wo = np.asarray(inputs["wo"], dtype=np.float32)
    fc = np.asarray(inputs["freqs_cos"], dtype=np.float32)
    fs = np.asarray(inputs["freqs_sin"], dtype=np.float32)
    mask = np.asarray(inputs["mask"], dtype=np.float32)

    xx = x.reshape(BS, DIM)
    # packed chunk-major: xT[p, tcn*(NKT*TCH) + k*TCH + t] = x[tcn*TCH+t, k*P+p]
    xT = np.ascontiguousarray(
        xx.reshape(NCH, TCH, NKT, P).transpose(3, 0, 2, 1)
        .reshape(P, NCH * NKT * TCH)).astype(NPBF)
    warm_x = np.ascontiguousarray(xx[0:TCH, 0:P].T).astype(NPBF)
    perm = np.concatenate([np.arange(0, HD, 2), np.arange(1, HD, 2)])
    wq_p = (wq.reshape(DIM, NH, HD)[:, :, perm] / math.sqrt(HD))
    wk_p = wk.reshape(DIM, NKV, HD)[:, :, perm]
    cosT = np.ascontiguousarray(fc.T).astype(NPBF)
    sinT = np.ascontiguousarray(fs.T).astype(NPBF)

    if not mask.any():
        variant = "none"
    else:
        il, jl = np.tril_indices(S)
        iu, ju = np.triu_indices(S, 1)
        if np.all(mask[il, jl] == 0.0) and np.all(mask[iu, ju] <= -1e8):
            variant = "causal"
        else:
            variant = "general"

    tri128 = None
    emaskT = None
    if variant == "causal":
        t = np.arange(P)[:, None]
        q = np.arange(P)[None, :]
        tri128 = (q >= t).astype(NPBF)
    elif variant == "general":
        with np.errstate(under="ignore", over="ignore"):
            emaskT = np.ascontiguousarray(np.exp(mask).T).astype(NPBF)

    nc = _get_prog(variant)

    in_maps = []
    for c in range(NCORES):
        wqc = wq_p[:, c * QH:(c + 1) * QH, :]                    # [DIM,QH,HD]
        wqc = np.ascontiguousarray(
            wqc.reshape(NKT, P, QH, HD).transpose(1, 2, 0, 3)
            .reshape(P, QH * NKT * HD)).astype(NPBF)
        wkc = np.ascontiguousarray(
            wk_p[:, c, :].reshape(NKT, P, HD).transpose(1, 0, 2)
            .reshape(P, NKT * HD)).astype(NPBF)
        wvc = np.ascontiguousarray(
            wv[:, c * HD:(c + 1) * HD].reshape(NKT, P, HD).transpose(1, 0, 2)
            .reshape(P, NKT * HD)).astype(NPBF)
        warm = np.concatenate(
            [warm_x]
            + [wqc[:, m * NKT * HD:m * NKT * HD + HD] for m in range(QH)]
            + [wkc[:, 0:HD], wvc[:, 0:HD]], axis=1)
        m = {
            "xT": xT,
            "warm": np.ascontiguousarray(warm).astype(NPBF),
            "wq": wqc,
            "wk": wkc,
            "wv": wvc,
            "wo": np.ascontiguousarray(
                wo[c * DQ:(c + 1) * DQ, :]).astype(NPBF),
            "cosT": cosT,
            "sinT": sinT,
        }
        if variant == "causal":
            m["tri"] = tri128
        elif variant == "general":
            m["emaskT"] = emaskT
        in_maps.append(m)
    return variant, nc, in_maps


def kernel(**inputs):
    global LAST_RESULTS
    variant, nc, in_maps = prepare(inputs)
    out = None
    for attempt in range(3):
        res = run_bass_kernel_spmd(nc, in_maps, core_ids=list(range(NCORES)))
        LAST_RESULTS = res
        out = np.zeros((BS, DIM), dtype=np.float64)
        ok = True
        for c in range(NCORES):
            p = np.asarray(res.results[c]["part"], dtype=np.float64)
            # flaky-execution guard: a healthy partial is finite, O(1)-scale,
            # and every token row is nonzero (dense projection of dense
            # data); huge values, NaNs, or any all-zero row mean the device
            # produced a bad/partial result -> re-execute
            rowmax = np.abs(p).max(axis=1)
            if not np.isfinite(p).all() or p.max() > 1e3 \
                    or p.min() < -1e3 or rowmax.min() == 0.0:
                ok = False
            out += p
        if ok:
            break
    return out.reshape(B, S, DIM).astype(np.float32)
